# revision 1
# baseline (speedup 1.0000x reference)
"""Distributed causal self-attention kernel for 8 Trainium2 NeuronCores.

Problem: B=2, T=2048, C=1024, H=16 heads, D=64 head dim.
    qkv = x @ wqkv.T; q,k = rmsnorm(q|k)*w; rope; causal attention; out @ wo.T

Sharding: core c handles batch b = c//4 and head group g = c%4 (4 heads).
Per core:
  - QKV projection for its (b, heads) in transposed [o, t] layout (q, k)
    plus natural [s, d] layout for v.
  - RMSNorm across d (partition dim) via a block-diagonal ones-matmul that
    sums and broadcasts in one shot, rsqrt on the scalar engine, RoPE as
    elementwise mul/add against host-precomputed cos/sin tables.
  - Causal attention per head in S^T = [s, t] layout.  ||q||=||k||=sqrt(D)
    after rmsnorm, so scores are bounded by D/sqrt(D)=8 and softmax needs
    no max subtraction.  The softmax denominator falls out of the AV matmul
    for free via a ones column appended to V (M=65).
  - ONE AllToAll (8 cores) swaps head-shards for T-shards; each core then
    owns t-range [256c, 256c+256) of BOTH batches and runs the wo
    projection against the full 16-head activation with no further
    reduction.

Matmul operands are float16 (fp32 PSUM accumulation).  The q tile for each
head is zero-padded to 128 partitions so every matmul contracts over K=128.
"""

import numpy as np

import concourse.bass as bass
import concourse.mybir as mybir
import concourse.tile as tile
from concourse import bacc
from concourse.bass_utils import run_bass_kernel_spmd

N_CORES = 8
B, T, C = 2, 2048, 1024
H, D = 16, 64          # global heads, head dim
HL = 4                 # heads per core
ROPE_THETA = 10000.0
EPS = 1e-6
KO = C // 128          # contraction chunks for C
NT = T // 512          # 512-wide t-chunks
NS = T // 128          # 128-wide s-chunks
TS = T // N_CORES      # t-rows owned per core after AllToAll (256)

F16 = mybir.dt.float16
F32 = mybir.dt.float32

_BUILD_CACHE = {}
SKIP_COLLECTIVE = False  # drop the AllToAll (for single-core TimelineSim)
N_COLLECTIVES = 1


def _build_program():
    nc = bacc.Bacc(
        "TRN2",
        target_bir_lowering=False,
        debug=False,
        enable_asserts=False,
        num_devices=N_CORES,
    )
    xT = nc.dram_tensor("xT", [C, T], F16, kind="ExternalInput").ap()
    wqkvT = nc.dram_tensor("wqkvT", [C, 3 * HL * D], F16, kind="ExternalInput").ap()
    woT = nc.dram_tensor("woT", [H * D, C], F16, kind="ExternalInput").ap()
    cos2 = nc.dram_tensor("cos2", [128, T], F16, kind="ExternalInput").ap()
    sin2 = nc.dram_tensor("sin2", [128, T], F16, kind="ExternalInput").ap()
    # rsqrt scale/bias with the norm weight folded in: rsqrt((sum*qkw_s + qkw_b))
    # == w * rsqrt(mean + eps) for w > 0
    qkw_s = nc.dram_tensor("qkw_s", [128, 2], F32, kind="ExternalInput").ap()
    qkw_b = nc.dram_tensor("qkw_b", [128, 2], F32, kind="ExternalInput").ap()
    onesseg = nc.dram_tensor("onesseg", [128, 128], F16, kind="ExternalInput").ap()
    masks = nc.dram_tensor("masks", [128, 4, 512], F16, kind="ExternalInput").ap()
    out = nc.dram_tensor("out", [B, TS, C], F32, kind="ExternalOutput").ap()

    with tile.TileContext(nc) as tc:
        _emit(tc, xT, wqkvT, woT, cos2, sin2, qkw_s, qkw_b, onesseg, masks, out)
    nc.compile()
    return nc


def _act_raw(eng, out, in_, func, bias=0.0, scale=1.0):
    """InstActivation without the bass-level Rsqrt ban (the table's measured
    error is ~4e-5 rel on hardware, fine for a norm scale)."""
    if not isinstance(bias, bass.AP):
        bias = eng.bass.const_aps.scalar_like(bias, in_)
    inputs = [eng.lower_ap(in_)]
    for arg in (bias, scale, 0.0):
        if isinstance(arg, bass.AP):
            inputs.append(eng.lower_ap(arg))
        else:
            inputs.append(mybir.ImmediateValue(dtype=mybir.dt.float32, value=arg))
    return eng.add_instruction(
        mybir.InstActivation(
            name=eng.bass.get_next_instruction_name(),
            func=func,
            ins=inputs,
            outs=[eng.lower_ap(out)],
        )
    )


def _emit(tc, xT, wqkvT, woT, cos2, sin2, qkw_s, qkw_b, onesseg, masks, out):
    nc = tc.nc
    Exp = mybir.ActivationFunctionType.Exp
    Rsqrt = mybir.ActivationFunctionType.Rsqrt

    import contextlib

    with contextlib.ExitStack() as ctx:
        const = ctx.enter_context(tc.tile_pool(name="const", bufs=1))
        work = ctx.enter_context(tc.tile_pool(name="work", bufs=3))
        work3 = ctx.enter_context(tc.tile_pool(name="work3", bufs=4))
        espool = ctx.enter_context(tc.tile_pool(name="espool", bufs=4))
        ps512 = ctx.enter_context(tc.tile_pool(name="ps512", bufs=2, space="PSUM"))
        pss2 = ctx.enter_context(tc.tile_pool(name="pss2", bufs=2, space="PSUM"))
        pso = ctx.enter_context(tc.tile_pool(name="pso", bufs=2, space="PSUM"))
        dram = ctx.enter_context(tc.tile_pool(name="dram", bufs=1, space="DRAM"))

        # ---- persistent SBUF tiles ----
        xT_sb = const.tile([128, KO, T], F16, tag="xT_sb")
        wqkvT_sb = const.tile([128, KO, 3 * HL * D], F16, tag="wqkvT_sb")
        woT_sb = const.tile([128, KO, C], F16, tag="woT_sb")
        cos2_sb = const.tile([128, T], F16, tag="cos2_sb")
        sin2_sb = const.tile([128, T], F16, tag="sin2_sb")
        qkws_sb = const.tile([128, 2], F32, tag="qkws_sb")
        qkwb_sb = const.tile([128, 2], F32, tag="qkwb_sb")
        ones_sb = const.tile([128, 128], F16, tag="ones_sb")
        masks_sb = const.tile([128, 4, 512], F16, tag="masks_sb")
        qT = [
            const.tile([128, T], F16, tag=f"qT{h}", name=f"qT{h}") for h in range(HL)
        ]
        kT = [
            const.tile([128, T], F16, tag=f"kT{p}", name=f"kT{p}") for p in range(2)
        ]
        vt = const.tile([128, NS, HL * (D + 1)], F16, tag="vt")
        aT = const.tile([128, 2 * KO, TS], F16, tag="aT")

        # single collective buffer: part 0 carries local heads {0,1}, 1 {2,3}
        cc = dram.tile([N_CORES, 2, 2 * D, TS], F16, tag="cc")
        cc_o = dram.tile([N_CORES, 2, 2 * D, TS], F16, tag="cc_o")

        # ---- input DMAs (small tables right after the first x/w chunks;
        # woT last -- it is only needed by the output projection) ----
        xT_r = xT.rearrange("(ko p) t -> p ko t", p=128)
        wq_r = wqkvT.rearrange("(ko p) o -> p ko o", p=128)
        # chunk-0's projections need only the first 512 t-columns of x, so
        # deliver exactly those (plus the weights) first; the rest of x, the
        # rope/norm tables, and woT stream in behind them
        for ko in range(KO):
            nc.sync.dma_start(xT_sb[:, ko, 0:512], xT_r[:, ko, 0:512])
            nc.sync.dma_start(wqkvT_sb[:, ko, :], wq_r[:, ko, :])
        nc.sync.dma_start(qkws_sb[:], qkw_s[:])
        nc.sync.dma_start(qkwb_sb[:], qkw_b[:])
        nc.sync.dma_start(ones_sb[:], onesseg[:])
        nc.sync.dma_start(cos2_sb[:, 0:512], cos2[:, 0:512])
        nc.sync.dma_start(sin2_sb[:, 0:512], sin2[:, 0:512])
        nc.sync.dma_start(masks_sb[:], masks[:])
        for ko in range(KO):
            nc.sync.dma_start(xT_sb[:, ko, 512:1024], xT_r[:, ko, 512:1024])
        nc.sync.dma_start(cos2_sb[:, 512:T], cos2[:, 512:T])
        nc.sync.dma_start(sin2_sb[:, 512:T], sin2[:, 512:T])
        for ko in range(KO):
            nc.sync.dma_start(xT_sb[:, ko, 1024:T], xT_r[:, ko, 1024:T])
        nc.sync.dma_start(woT_sb[:], woT.rearrange("(ko p) e -> p ko e", p=128))

        # zero the unused halves of the per-head padded q tiles
        nc.vector.memset(qT[0][64:128, :], 0.0)
        nc.vector.memset(qT[2][64:128, :], 0.0)
        nc.vector.memset(qT[1][0:64, :], 0.0)
        nc.vector.memset(qT[3][0:64, :], 0.0)
        # ones column in the V tiles (drives the softmax denominator)
        nc.vector.memset(
            vt[:].rearrange("p s (h c) -> p s h c", c=D + 1)[:, :, :, D : D + 1], 1.0
        )

        def emit_qk(ot, ti):
            """q/k projection + rmsnorm + rope for one head-pair o-tile and
            one 512-wide t-chunk."""
            is_q = ot < 2
            pair = ot % 2
            wcol = 0 if is_q else 1
            if True:
                tsl = bass.ts(ti, 512)
                ps = ps512.tile([128, 512], F32, tag="ps512", name="ps")
                for ko in range(KO):
                    nc.tensor.matmul(
                        ps[:],
                        lhsT=wqkvT_sb[:, ko, bass.ts(ot, 128)],
                        rhs=xT_sb[:, ko, tsl],
                        start=(ko == 0),
                        stop=(ko == KO - 1),
                    )
                # Evacuate psum immediately (frees the bank for the next
                # accumulation) and run the chain from the f16 copy.
                rawe = work3.tile([128, 512], F16, tag="rawe", name="rawe")
                nc.scalar.copy(rawe[:], ps[:])
                # Per-head sum of squares, replicated across the head's 64
                # partitions by the block-diagonal ones lhsT in one matmul
                # (partition_broadcast with non-zero partition offsets is
                # broken on hardware).  The norm weight is folded into the
                # rsqrt's per-partition scale/bias.
                sq = work.tile([128, 512], F16, tag="sq", name="sq")
                nc.vector.tensor_mul(sq[:], rawe[:], rawe[:])
                ms = ps512.tile([128, 512], F32, tag="ps512", name="ms")
                nc.tensor.matmul(
                    ms[:], lhsT=ones_sb[:], rhs=sq[:], start=True, stop=True
                )
                cb = work.tile([128, 512], F32, tag="cb", name="cb")
                _act_raw(
                    nc.scalar,
                    cb[:],
                    ms[:],
                    Rsqrt,
                    bias=qkwb_sb[:, wcol : wcol + 1],
                    scale=qkws_sb[:, wcol : wcol + 1],
                )
                raw = work3.tile([128, 512], F16, tag="raw", name="raw")
                nc.vector.tensor_mul(raw[:], rawe[:], cb[:])
                # rope (f16 chain): the sign-interleaved sin table lets the
                # partner-half products write rot directly, no shuffle copies
                rot = work.tile([128, 512], F16, tag="rot", name="rot")
                sl = sin2_sb[:, tsl]
                nc.vector.tensor_mul(rot[0:32, :], raw[32:64, :], sl[32:64, :])
                nc.vector.tensor_mul(rot[32:64, :], raw[0:32, :], sl[0:32, :])
                nc.vector.tensor_mul(rot[64:96, :], raw[96:128, :], sl[96:128, :])
                nc.vector.tensor_mul(rot[96:128, :], raw[64:96, :], sl[64:96, :])
                nc.vector.tensor_mul(raw[:], raw[:], cos2_sb[:, tsl])
                if is_q:
                    h0, h1 = 2 * pair, 2 * pair + 1
                    nc.vector.tensor_add(
                        qT[h0][0:64, tsl], raw[0:64, :], rot[0:64, :]
                    )
                    nc.vector.tensor_add(
                        qT[h1][64:128, tsl], raw[64:128, :], rot[64:128, :]
                    )
                else:
                    nc.vector.tensor_add(kT[pair][:, tsl], raw[:], rot[:])

        def emit_v(st):
            vt_heads = vt[:].rearrange("p s (h c) -> p s h c", c=D + 1)
            if True:
                psv = ps512.tile([128, 512], F32, tag="ps512", name="psv")
                for ko in range(KO):
                    nc.tensor.matmul(
                        psv[:, 0 : HL * D],
                        lhsT=xT_sb[:, ko, bass.ts(st, 128)],
                        rhs=wqkvT_sb[:, ko, 2 * HL * D : 3 * HL * D],
                        start=(ko == 0),
                        stop=(ko == KO - 1),
                    )
                nc.scalar.copy(
                    vt_heads[:, st, :, 0:D],
                    psv[:, 0 : HL * D].rearrange("p (h c) -> p h c", c=D),
                )

        def emit_attn(h, ti):
            """attention for one head and one 512-wide t-chunk; s-chunks run in pairs sharing a
            [128, 1024] psum tile so one exp / mask covers two blocks."""
            pair = h // 2
            part = 0 if h < 2 else 1
            hrow = bass.ts(h % 2, D)
            if True:
                po = pso.tile([D + 1, 512], F32, tag="pso", name="po")
                n_si = 4 * ti + 4
                for si0 in range(0, n_si, 2):
                    j = si0 - 4 * ti
                    # the last (most-diagonal) pair only attends within the
                    # upper half of the t-chunk -- the lower half is fully
                    # masked zeros, so compute it at half width
                    last = j == 2
                    tw = 256 if last else 512
                    toff = 512 * ti + (512 - tw)
                    ps2 = pss2.tile([128, 2, 512], F32, tag="pss2", name="ps2")
                    for u in range(2):
                        nc.tensor.matmul(
                            ps2[:, u, 0:tw],
                            lhsT=kT[pair][:, bass.ts(si0 + u, 128)],
                            rhs=qT[h][:, bass.ds(toff, tw)],
                            start=True,
                            stop=True,
                        )
                    es = espool.tile([128, 2, 512], F16, tag="es", name="es")
                    nc.scalar.activation(
                        es[:, :, 0:tw], ps2[:, :, 0:tw], Exp, scale=1.0 / np.sqrt(D)
                    )
                    if j >= 0:
                        nc.vector.tensor_mul(
                            es[:, :, 0:tw],
                            es[:, :, 0:tw],
                            masks_sb[:, j : j + 2, 512 - tw : 512],
                        )
                    for u in range(2):
                        nc.tensor.matmul(
                            po[:, 512 - tw : 512],
                            lhsT=vt[:, si0 + u, bass.ds(h * (D + 1), D + 1)],
                            rhs=es[:, u, 0:tw],
                            start=(si0 + u == 0),
                            stop=(si0 + u == n_si - 1),
                        )
                rec = work.tile([1, 512], F32, tag="rec", name="rec")
                nc.vector.reciprocal(rec[:], po[D : D + 1, :])
                rb = work.tile([64, 512], F32, tag="rb", name="rb")
                nc.gpsimd.partition_broadcast(rb[:], rec[:])
                osb = work.tile([64, 512], F16, tag="osb", name="osb")
                nc.vector.tensor_mul(osb[:], po[0:D, :], rb[:])
                nc.sync.dma_start(cc[2 * ti, part, hrow, :], osb[:, 0:TS])
                nc.sync.dma_start(cc[2 * ti + 1, part, hrow, :], osb[:, TS:512])

        def emit_a2a():
            if SKIP_COLLECTIVE:
                return
            nc.gpsimd.collective_compute(
                "AllToAll",
                mybir.AluOpType.bypass,
                replica_groups=[list(range(N_CORES))],
                ins=[cc.opt()],
                outs=[cc_o.opt()],
            )

        def emit_at_loads():
            # aT column b*KO+ko holds hd rows [128*ko, 128*ko+128) of batch b
            # = global heads {2*ko, 2*ko+1}: source core 4b + ko//2, part ko%2
            for b in range(B):
                for ko in range(KO):
                    nc.sync.dma_start(
                        aT[:, b * KO + ko, :], cc_o[4 * b + ko // 2, ko % 2, :, :]
                    )

        def emit_outproj():
            for b in range(B):
                for tt in range(TS // 128):
                    for ec in range(C // 512):
                        pout = ps512.tile([128, 512], F32, tag="ps512", name="pout")
                        for ko in range(KO):
                            nc.tensor.matmul(
                                pout[:],
                                lhsT=aT[:, b * KO + ko, bass.ts(tt, 128)],
                                rhs=woT_sb[:, ko, bass.ts(ec, 512)],
                                start=(ko == 0),
                                stop=(ko == KO - 1),
                            )
                        ob = work.tile([128, 512], F32, tag="ob", name="ob")
                        nc.vector.tensor_copy(ob[:], pout[:])
                        nc.sync.dma_start(
                            out[b, bass.ts(tt, 128), bass.ts(ec, 512)], ob[:]
                        )

        # ---- emission order: chunk-0 projections up front, then per
        # 512-wide t-chunk the NEXT chunk's projection work is round-robined
        # into the attention head loop (one o-tile + one v s-chunk per head
        # slot) so the scalar engine's exp stream overlaps the projections
        # instead of bunching after them; then ONE AllToAll and the output
        # projection ----
        ot_order = (0, 2, 1, 3)
        for ot in ot_order:
            emit_qk(ot, 0)
        for st in range(4):
            emit_v(st)
        for ti in range(NT):
            for h in range(HL):
                emit_attn(h, ti)
                if ti + 1 < NT:
                    emit_qk(ot_order[h], ti + 1)
                    emit_v(4 * (ti + 1) + h)
        emit_a2a()
        emit_at_loads()
        emit_outproj()


def _host_inputs(x, wqkv, wo, q_norm_w, k_norm_w):
    """Build the per-core input maps (all host-side prep is layout/dtype only)."""
    x = np.asarray(x, dtype=np.float32)
    wqkv = np.asarray(wqkv, dtype=np.float32)
    wo = np.asarray(wo, dtype=np.float32)
    q_norm_w = np.asarray(q_norm_w, dtype=np.float32)
    k_norm_w = np.asarray(k_norm_w, dtype=np.float32)

    # rope tables, f32 arithmetic to match the reference
    inv_freq = (1.0 / (ROPE_THETA ** (np.arange(0, D, 2, dtype=np.float32) / D))).astype(
        np.float32
    )
    freqs = np.arange(T, dtype=np.float32)[:, None] * inv_freq[None, :]  # [T, 32]
    cosT = np.cos(freqs).T.astype(np.float32)  # [32, T]
    sinT = np.sin(freqs).T.astype(np.float32)
    cos2 = np.ascontiguousarray(np.tile(cosT, (4, 1))).astype(np.float16)  # [128, T]
    # sign-interleaved: row block b holds (+sinT if b even else -sinT); the
    # rope kernel reads the PARTNER half's rows, so out[0:32] picks up block 1
    # (-sinT) etc., matching x1*cos - x2*sin / x1*sin + x2*cos
    sin2 = np.ascontiguousarray(
        np.concatenate([sinT, -sinT, sinT, -sinT], axis=0)
    ).astype(np.float16)

    qw2 = np.concatenate([q_norm_w, q_norm_w])  # [128]
    kw2 = np.concatenate([k_norm_w, k_norm_w])
    qkw_s = np.stack(
        [1.0 / (D * qw2 * qw2), 1.0 / (D * kw2 * kw2)], axis=1
    ).astype(np.float32)  # [128, 2]
    qkw_b = np.stack(
        [EPS / (qw2 * qw2), EPS / (kw2 * kw2)], axis=1
    ).astype(np.float32)

    onesseg = np.zeros((128, 128), dtype=np.float16)
    onesseg[0:64, 0:64] = 1.0
    onesseg[64:128, 64:128] = 1.0

    p = np.arange(128)[:, None, None]
    jj = np.arange(4)[None, :, None]
    tp = np.arange(512)[None, None, :]
    masks = (128 * jj + p <= tp).astype(np.float16)  # [128, 4, 512]

    woT = np.ascontiguousarray(wo.T).astype(np.float16)  # [hd, e]

    xT_b = [np.ascontiguousarray(x[b].T).astype(np.float16) for b in range(B)]

    in_maps = []
    for c in range(N_CORES):
        b, g = c // 4, c % 4
        rq = slice(256 * g, 256 * g + 256)
        wsel = np.concatenate(
            [wqkv[rq], wqkv[C:][rq], wqkv[2 * C :][rq]], axis=0
        )  # [768, C]
        wqkvT = np.ascontiguousarray(wsel.T).astype(np.float16)
        in_maps.append(
            {
                "xT": xT_b[b],
                "wqkvT": wqkvT,
                "woT": woT,
                "cos2": cos2,
                "sin2": sin2,
                "qkw_s": qkw_s,
                "qkw_b": qkw_b,
                "onesseg": onesseg,
                "masks": masks,
            }
        )
    return in_maps


def get_program():
    if "nc" not in _BUILD_CACHE:
        _BUILD_CACHE["nc"] = _build_program()
    return _BUILD_CACHE["nc"]


def kernel(x, wqkv, wo, q_norm_w, k_norm_w):
    nc = get_program()
    in_maps = _host_inputs(x, wqkv, wo, q_norm_w, k_norm_w)
    res = run_bass_kernel_spmd(nc, in_maps, core_ids=list(range(N_CORES)))
    full = np.empty((B, T, C), dtype=np.float32)
    for c in range(N_CORES):
        o = res.results[c]["out"]  # [B, TS, C]
        full[:, TS * c : TS * (c + 1), :] = o
    return full



# revision 23
# speedup vs baseline: 1.0953x; 1.0953x over previous
"""Distributed causal self-attention kernel for 8 Trainium2 NeuronCores.

Problem: B=2, T=2048, C=1024, H=16 heads, D=64 head dim.
    qkv = x @ wqkv.T; q,k = rmsnorm(q|k)*w; rope; causal attention; out @ wo.T

Sharding: core c handles batch b = c//4 and head group g = c%4 (4 heads).
Per core:
  - QKV projection for its (b, heads) in transposed [o, t] layout (q, k)
    plus natural [s, d] layout for v.  q and k are both packed 2 heads per
    128-partition tile; the per-head QK matmul contracts over K=64 via
    base-partition-64 operand slices (no zero padding).
  - RMSNorm across d via a block-diagonal ones-matmul that sums and
    broadcasts per head in one shot.  rsqrt is computed as exp(-0.5*ln(y))
    so every scalar-engine activation (Exp/Ln/Copy) lives in ONE hardware
    table set and no table reloads occur.
  - RoPE via a single cross-partition stream_shuffle: the head dim is laid
    out host-side so each rope partner pair sits 16 partitions apart inside
    one 32-partition quadrant; the sign-folded sin table makes
    out = x*cos + shuffle(x)*sin exact.
  - Causal attention per head in S^T = [s, t] layout.  ||q||=||k||=sqrt(D)
    after rmsnorm, so scores are bounded and softmax needs no max
    subtraction.  exp runs on psum score pairs; only true diagonal
    128x128 blocks are masked, by one triangular table on the (otherwise
    idle) gpsimd engine.
  - AV runs TRANSPOSED: for each 128-wide t-block, matmul(lhsT=es_block,
    rhs=v_chunk) accumulates [t, d+1] in psum with N=65 per matmul --
    about half the tensor-engine streaming cost of the [d, t] form.  The
    ones column of V gives the softmax denominator; the division is a
    per-partition reciprocal + broadcast multiply during psum evacuation.
  - ONE AllToAll (8 cores) swaps head-shards for T-shards in [t, h, d]
    layout; each core then DMA-TRANSPOSES (xbar dma transpose, ~14ns/tile)
    its received blocks into [hd, t] tiles and runs the wo projection with
    no further reduction.

Matmul operands are float16 (fp32 PSUM accumulation).
"""

import numpy as np

import concourse.bass as bass
import concourse.mybir as mybir
import concourse.tile as tile
from concourse import bacc
from concourse.bass_utils import run_bass_kernel_spmd

N_CORES = 8
B, T, C = 2, 2048, 1024
H, D = 16, 64          # global heads, head dim
HL = 4                 # heads per core
ROPE_THETA = 10000.0
EPS = 1e-6
KO = C // 128          # contraction chunks for C
NT = T // 512          # 512-wide t-chunks
NS = T // 128          # 128-wide s-chunks
TS = T // N_CORES      # t-rows owned per core after AllToAll (256)

F16 = mybir.dt.float16
F32 = mybir.dt.float32

_BUILD_CACHE = {}
SKIP_COLLECTIVE = False  # drop the AllToAll (for single-core TimelineSim)
N_COLLECTIVES = 1

# rope partner shuffle: swap 16-partition halves within each 32-partition
# quadrant (see _host_inputs for the matching weight-row interleave)
SWAP16 = [(i + 16) % 32 for i in range(32)]


def _build_program():
    nc = bacc.Bacc(
        "TRN2",
        target_bir_lowering=False,
        debug=False,
        enable_asserts=False,
        num_devices=N_CORES,
    )
    xT = nc.dram_tensor("xT", [C, T], F16, kind="ExternalInput").ap()
    wqkvT = nc.dram_tensor("wqkvT", [C, 3 * HL * D], F16, kind="ExternalInput").ap()
    woT = nc.dram_tensor("woT", [H * D, C], F16, kind="ExternalInput").ap()
    # rope tables: [:, 0] = cos, [:, 1] = sign-folded sin
    cossin = nc.dram_tensor("cossin", [128, 2, T], F16, kind="ExternalInput").ap()
    # ln scale/bias with the norm weight folded in ([:, 0:2] = scale q/k,
    # [:, 2:4] = bias q/k):
    # exp(-0.5*ln(sum*qkw_s + qkw_b)) == w * rsqrt(mean + eps) for w > 0
    qkw = nc.dram_tensor("qkw", [128, 4], F32, kind="ExternalInput").ap()
    # [:, 0:128] block-diag ones (rmsnorm sum), [:, 128:256] causal triangle
    tables = nc.dram_tensor("tables", [128, 256], F16, kind="ExternalInput").ap()
    out = nc.dram_tensor("out", [B, TS, C], F16, kind="ExternalOutput").ap()

    with tile.TileContext(nc) as tc:
        _emit(tc, xT, wqkvT, woT, cossin, qkw, tables, out)
    nc.compile()
    return nc


def _emit(tc, xT, wqkvT, woT, cossin, qkw, tables, out):
    nc = tc.nc
    Exp = mybir.ActivationFunctionType.Exp
    Ln = mybir.ActivationFunctionType.Ln

    import contextlib

    with contextlib.ExitStack() as ctx:
        const = ctx.enter_context(tc.tile_pool(name="const", bufs=1))
        work = ctx.enter_context(tc.tile_pool(name="work", bufs=3))
        work3 = ctx.enter_context(tc.tile_pool(name="work3", bufs=4))
        espool = ctx.enter_context(tc.tile_pool(name="espool", bufs=4))
        ps512 = ctx.enter_context(tc.tile_pool(name="ps512", bufs=2, space="PSUM"))
        pss2 = ctx.enter_context(tc.tile_pool(name="pss2", bufs=2, space="PSUM"))
        po4p = ctx.enter_context(tc.tile_pool(name="po4p", bufs=2, space="PSUM"))
        dram = ctx.enter_context(tc.tile_pool(name="dram", bufs=1, space="DRAM"))

        # ---- persistent SBUF tiles ----
        xT_sb = const.tile([128, KO, T], F16, tag="xT_sb")
        wqkvT_sb = const.tile([128, KO, 3 * HL * D], F16, tag="wqkvT_sb")
        woT_sb = const.tile([128, KO, C], F16, tag="woT_sb")
        cossin_sb = const.tile([128, 2, T], F16, tag="cossin_sb")
        qkw_sb = const.tile([128, 4], F32, tag="qkw_sb")
        tables_sb = const.tile([128, 256], F16, tag="tables_sb")
        cos2_sb = cossin_sb[:, 0, :]
        sin2_sb = cossin_sb[:, 1, :]
        ones_sb = tables_sb[:, 0:128]
        mtri_sb = tables_sb[:, 128:256]
        qT = [
            const.tile([128, T], F16, tag=f"qT{p}", name=f"qT{p}") for p in range(2)
        ]
        kT = [
            const.tile([128, T], F16, tag=f"kT{p}", name=f"kT{p}") for p in range(2)
        ]
        vt = const.tile([128, NS, HL * (D + 1)], F16, tag="vt")
        aT = const.tile([128, 2 * KO, TS], F16, tag="aT")

        # collective buffer in [t, h, d] layout:
        # (dst core, t-slot of 128, 128 t, 4 heads, 64 d)
        cc = dram.tile([N_CORES, 2, 128, HL, D], F16, tag="cc")
        cc_o = dram.tile([N_CORES, 2, 128, HL, D], F16, tag="cc_o")

        # ---- input DMAs (batched to keep HWDGE serialization low; the
        # first 512 t-columns of x and the qkv weights come first, small
        # tables next, then the rest of x; woT last -- it is only needed
        # by the output projection) ----
        xT_r = xT.rearrange("(ko p) t -> p ko t", p=128)
        wq_r = wqkvT.rearrange("(ko p) o -> p ko o", p=128)
        # first 512 t-columns of x and the weights in 2-ko chunks so the
        # in-order PE can start the first projection accumulation while the
        # rest streams in
        for koq in range(4):
            ks = bass.ts(koq, 2)
            nc.sync.dma_start(xT_sb[:, ks, 0:512], xT_r[:, ks, 0:512])
            nc.sync.dma_start(wqkvT_sb[:, ks, :], wq_r[:, ks, :])
        nc.sync.dma_start(qkw_sb[:], qkw[:])
        nc.sync.dma_start(tables_sb[:], tables[:])
        nc.sync.dma_start(cossin_sb[:, :, 0:512], cossin[:, :, 0:512])
        for koh in range(2):
            ks = bass.ts(koh, 4)
            nc.sync.dma_start(xT_sb[:, ks, 512:1024], xT_r[:, ks, 512:1024])
        nc.sync.dma_start(cossin_sb[:, :, 512:T], cossin[:, :, 512:T])
        for koh in range(2):
            ks = bass.ts(koh, 4)
            nc.sync.dma_start(xT_sb[:, ks, 1024:T], xT_r[:, ks, 1024:T])
        nc.sync.dma_start(woT_sb[:], woT.rearrange("(ko p) e -> p ko e", p=128))

        # ones column in the V tiles (drives the softmax denominator)
        nc.vector.memset(
            vt[:].rearrange("p s (h c) -> p s h c", c=D + 1)[:, :, :, D : D + 1], 1.0
        )

        # Load activation-table set 6 (natural_log_exp_and_others) ONCE up
        # front: it contains every function this kernel uses (Exp, Ln, Copy,
        # Square), so the Bacc fixpoint pass never needs to insert another
        # (it would otherwise thrash between the ln-only and exp-only sets).
        nc.scalar.add_instruction(
            mybir.InstLoadActFuncSet(
                name=nc.get_next_instruction_name(),
                ins=[],
                outs=[],
                act_func_set_id=6,
            )
        )

        # ---- thunk machinery: the PE executes strictly in order, so small
        # PE work units (projection ko-chunks, the norm ones-matmul) are
        # queued and popped between attention pairs.  This fills the PE
        # while the scalar engine's exp gates each pair's AV matmuls, and
        # it naturally delays each norm matmul until its DVE-side sum of
        # squares is long done. ----
        from collections import deque

        filler = deque()

        def pop_filler(n=1):
            for _ in range(n):
                if filler:
                    filler.popleft()()

        def flush_filler():
            while filler:
                filler.popleft()()

        def qk_thunks(ot, ti):
            """q/k projection + rmsnorm + rope for one head-pair o-tile and
            one 512-wide t-chunk, split into 3 PE-granular thunks."""
            is_q = ot < 2
            pair = ot % 2
            wcol = 0 if is_q else 1
            dest = qT[pair] if is_q else kT[pair]
            tsl = bass.ts(ti, 512)
            st = {}

            def mk_proj(k0):
                def tp():
                    if k0 == 0:
                        st["ps"] = ps512.tile([128, 512], F32, tag="ps512", name="ps")
                    ps = st["ps"]
                    for ko in range(k0, k0 + 2):
                        nc.tensor.matmul(
                            ps[:],
                            lhsT=wqkvT_sb[:, ko, bass.ts(ot, 128)],
                            rhs=xT_sb[:, ko, tsl],
                            start=(ko == 0),
                            stop=(ko == KO - 1),
                        )
                    if k0 + 2 == KO:
                        # Evacuate psum immediately (frees the bank for the
                        # next accumulation), run the chain from the f16 copy.
                        rawe = work3.tile([128, 512], F16, tag="rawe", name="rawe")
                        nc.vector.tensor_copy(rawe[:], ps[:])
                        # rope partner shuffle does not depend on the norm
                        # coefficient (cb is constant across partner rows, so
                        # shuffle(raw*cb) == shuffle(raw)*cb and cb folds
                        # into the cos/sin coefficient tiles in t3).
                        rsw = work3.tile([128, 512], F16, tag="rsw", name="rsw")
                        nc.vector.stream_shuffle(rsw[:], rawe[:], SWAP16)
                        sq = work3.tile([128, 512], F16, tag="sq", name="sq")
                        nc.vector.tensor_mul(sq[:], rawe[:], rawe[:])
                        st["rawe"], st["rsw"], st["sq"] = rawe, rsw, sq

                return tp

            def t3():
                # Per-head sum of squares, replicated across the head's 64
                # partitions by the block-diagonal ones lhsT in one matmul.
                # The norm weight is folded into the ln's per-partition
                # scale/bias; rsqrt(y) = exp(-0.5*ln(y)) keeps the scalar
                # engine inside ONE activation-table set (Exp/Ln/Copy).
                ms = ps512.tile([128, 512], F32, tag="ps512", name="ms")
                nc.tensor.matmul(
                    ms[:], lhsT=ones_sb[:], rhs=st["sq"][:], start=True, stop=True
                )
                lncb = work.tile([128, 512], F32, tag="lncb", name="lncb")
                nc.scalar.activation(
                    lncb[:],
                    ms[:],
                    Ln,
                    bias=qkw_sb[:, 2 + wcol : 3 + wcol],
                    scale=qkw_sb[:, wcol : wcol + 1],
                )
                cb = work.tile([128, 512], F16, tag="cb", name="cb")
                nc.scalar.activation(cb[:], lncb[:], Exp, scale=-0.5)
                # dest = rawe*(cb*cos) + shuffle(rawe)*(cb*sin)
                cbc = work.tile([128, 512], F16, tag="cbc", name="cbc")
                nc.vector.tensor_mul(cbc[:], cb[:], cos2_sb[:, tsl])
                cbs = work.tile([128, 512], F16, tag="cbs", name="cbs")
                nc.vector.tensor_mul(cbs[:], cb[:], sin2_sb[:, tsl])
                rot = work.tile([128, 512], F16, tag="rot", name="rot")
                nc.vector.tensor_mul(rot[:], st["rsw"][:], cbs[:])
                cosm = work.tile([128, 512], F16, tag="cosm", name="cosm")
                nc.vector.tensor_mul(cosm[:], st["rawe"][:], cbc[:])
                nc.vector.tensor_add(dest[:, tsl], cosm[:], rot[:])

            return [mk_proj(0), mk_proj(2), mk_proj(4), mk_proj(6), t3]

        def v_thunks(st_):
            vt_heads = vt[:].rearrange("p s (h c) -> p s h c", c=D + 1)
            state = {}

            def mk_v(k0):
                def tv():
                    if k0 == 0:
                        state["psv"] = ps512.tile(
                            [128, 512], F32, tag="ps512", name="psv"
                        )
                    psv = state["psv"]
                    for ko in range(k0, k0 + 4):
                        nc.tensor.matmul(
                            psv[:, 0 : HL * D],
                            lhsT=xT_sb[:, ko, bass.ts(st_, 128)],
                            rhs=wqkvT_sb[:, ko, 2 * HL * D : 3 * HL * D],
                            start=(ko == 0),
                            stop=(ko == KO - 1),
                        )
                    if k0 + 4 == KO:
                        nc.vector.tensor_copy(
                            vt_heads[:, st_, :, 0:D],
                            psv[:, 0 : HL * D].rearrange("p (h c) -> p h c", c=D),
                        )

                return tv

            return [mk_v(0), mk_v(4)]

        def emit_attn(h, ti):
            """attention for one head and one 512-wide t-chunk.  Scores come
            in s-chunk pairs sharing a [128, 2, 512] psum tile (one exp per
            pair); AV accumulates transposed [t, d+1] tiles per 128-wide
            t-block with N=65 matmuls.  The pair loop is software-pipelined:
            pair p's AV runs after pair p+1's QK (plus one filler thunk), so
            the exp latency hides behind real PE work."""
            pair = h // 2
            hrow = bass.ds(64 * (h % 2), 64)
            po4 = po4p.tile([128, 4, D + 1], F32, tag="po4", name="po4")
            n_si = 4 * ti + 4

            def emit_av(p):
                (es, j, cut) = p
                # mask ONLY the true diagonal 128x128 blocks, on gpsimd
                for u in range(2):
                    tj = j + u
                    if tj >= 0:
                        blk = es[:, u, bass.ds(128 * tj - cut, 128)]
                        nc.gpsimd.tensor_mul(blk, blk, mtri_sb[:])
                # transposed AV: accumulate [t, d+1] per 128-wide t-block.
                # The whole po4 bank is ONE psum accumulation group (psum
                # group start/stop is bank-granular); per-element
                # has_written bits make the first write to each t-block an
                # overwrite and later ones accumulates.
                si0 = 4 * ti + j
                for u in range(2):
                    si = si0 + u
                    for tj in range(max(0, j + u), 4):
                        nc.tensor.matmul(
                            po4[:, tj, :],
                            lhsT=es[:, u, bass.ds(128 * tj - cut, 128)],
                            rhs=vt[:, si, bass.ds(h * (D + 1), D + 1)],
                            start=(si == 0 and tj == 0),
                            stop=(si == 4 * ti + 3 and tj == 3),
                        )

            prev = None
            for si0 in range(0, n_si, 2):
                j = si0 - 4 * ti
                # the last (most-diagonal) pair only attends within the
                # upper half of the t-chunk -- the lower half is fully
                # masked zeros, so compute it at half width
                last = j == 2
                tw = 256 if last else 512
                cut = 512 - tw           # es col 0 == chunk-local t = cut
                toff = 512 * ti + cut
                ps2 = pss2.tile([128, 2, 512], F32, tag="pss2", name="ps2")
                for u in range(2):
                    nc.tensor.matmul(
                        ps2[:, u, 0:tw],
                        lhsT=kT[pair][hrow, bass.ts(si0 + u, 128)],
                        rhs=qT[pair][hrow, bass.ds(toff, tw)],
                        start=True,
                        stop=True,
                    )
                es = espool.tile([128, 2, 512], F16, tag="es", name="es")
                nc.scalar.activation(
                    es[:, :, 0:tw], ps2[:, :, 0:tw], Exp, scale=1.0 / np.sqrt(D)
                )
                if prev is not None:
                    pop_filler()
                    emit_av(prev)
                prev = (es, j, cut)
            pop_filler()
            emit_av(prev)
            # softmax division on evacuation: per-partition reciprocal of
            # the ones-column, broadcast along d
            rec4 = work.tile([128, 4], F32, tag="rec4", name="rec4")
            nc.vector.reciprocal(
                rec4[:], po4[:, :, D : D + 1].rearrange("p a b -> p (a b)")
            )
            ob = work.tile([128, 4, D], F16, tag="ob", name="ob")
            nc.vector.tensor_mul(
                ob[:], po4[:, :, 0:D], rec4[:].broadcast_to([128, 4, D])
            )
            # one DMA: t-block tj goes to dst core (4*ti+tj)//2, slot
            # (4*ti+tj)%2 -- contiguous (dst, slot) blocks in cc
            ccr = cc[:].rearrange("dst slot t h d -> t (dst slot) h d")
            nc.sync.dma_start(ccr[:, bass.ds(4 * ti, 4), h, :], ob[:])

        def emit_a2a():
            if SKIP_COLLECTIVE:
                return
            nc.gpsimd.collective_compute(
                "AllToAll",
                mybir.AluOpType.bypass,
                replica_groups=[list(range(N_CORES))],
                ins=[cc.opt()],
                outs=[cc_o.opt()],
            )

        def emit_tail():
            # aT column b*KO+ko holds hd rows [128*ko, 128*ko+128) of batch
            # b = global heads {2*ko, 2*ko+1}: source core 4*b + g covers
            # columns b*KO + {2g, 2g+1}.  One xbar dma transpose per
            # (source core, t-slot) turns the received [128 t, 4 h, 64 d]
            # block into both aT column tiles at once; the out projection
            # for that (b, t-slot) follows immediately so PE work overlaps
            # the remaining transposes.  In SKIP_COLLECTIVE timing mode the
            # transposes read cc itself, which models the true
            # after-all-attention ordering without the collective.
            src_buf = cc if SKIP_COLLECTIVE else cc_o
            for b in range(B):
                for slot in range(2):
                    for g in range(4):
                        nc.sync.dma_start_transpose(
                            aT[:, bass.ds(b * KO + 2 * g, 2), bass.ds(128 * slot, 128)],
                            src_buf[4 * b + g, slot, :, :, :],
                        )
                    tt = slot
                    for ec in range(C // 512):
                        pout = ps512.tile([128, 512], F32, tag="ps512", name="pout")
                        for ko in range(KO):
                            nc.tensor.matmul(
                                pout[:],
                                lhsT=aT[:, b * KO + ko, bass.ts(tt, 128)],
                                rhs=woT_sb[:, ko, bass.ts(ec, 512)],
                                start=(ko == 0),
                                stop=(ko == KO - 1),
                            )
                        ob = work.tile([128, 512], F16, tag="obp", name="obp")
                        nc.vector.tensor_copy(ob[:], pout[:])
                        nc.sync.dma_start(
                            out[b, bass.ts(tt, 128), bass.ts(ec, 512)], ob[:]
                        )

        # ---- emission order: chunk-0 projections up front (norm matmuls
        # staggered one slot behind their projections), then per 512-wide
        # t-chunk the NEXT chunk's projection thunks are queued and popped
        # between attention pairs; then ONE AllToAll, the dma-transposed
        # aT loads, and the output projection ----
        ot_order = (0, 2, 1, 3)
        pre = [qk_thunks(ot, 0) for ot in ot_order]
        vpre = [v_thunks(st) for st in range(4)]
        for i in range(4):
            for tp in pre[i][:4]:
                tp()
            if i > 0:
                pre[i - 1][4]()
            vpre[i][0]()
            vpre[i][1]()
        pre[3][4]()
        for ti in range(NT):
            # queue the next chunk's projection thunks; attention pairs pop
            # one between each score/AV step so the PE never idles while the
            # scalar engine's exp stream gates the AVs
            if ti + 1 < NT:
                for h in range(HL):
                    filler.extend(qk_thunks(ot_order[h], ti + 1))
                    filler.extend(v_thunks(4 * (ti + 1) + h))
            for h in range(HL):
                emit_attn(h, ti)
            flush_filler()
        emit_a2a()
        emit_tail()


def _host_inputs(x, wqkv, wo, q_norm_w, k_norm_w):
    """Build the per-core input maps (all host-side prep is layout/dtype only)."""
    x = np.asarray(x, dtype=np.float32)
    wqkv = np.asarray(wqkv, dtype=np.float32)
    wo = np.asarray(wo, dtype=np.float32)
    q_norm_w = np.asarray(q_norm_w, dtype=np.float32)
    k_norm_w = np.asarray(k_norm_w, dtype=np.float32)

    # head-dim interleave: rope partner pairs (x1_i, x2_i) sit 16 partitions
    # apart within one 32-partition quadrant, so ONE stream_shuffle (swap
    # 16-halves per quadrant) aligns every partner.  perm64[new] = old.
    perm64 = np.concatenate(
        [np.arange(0, 16), np.arange(32, 48), np.arange(16, 32), np.arange(48, 64)]
    )
    # rope frequency index and partner-sign per (new) position
    j = np.arange(64)
    q32, r32 = j // 32, j % 32
    freq_idx = 16 * q32 + (r32 % 16)          # pair index i in [0, 32)
    is_x1 = r32 < 16                          # rows holding x1_i

    inv_freq = (
        1.0 / (ROPE_THETA ** (np.arange(0, D, 2, dtype=np.float32) / D))
    ).astype(np.float32)
    freqs = np.arange(T, dtype=np.float32)[:, None] * inv_freq[None, :]  # [T, 32]
    cosT = np.cos(freqs).astype(np.float32)  # [T, 32]
    sinT = np.sin(freqs).astype(np.float32)
    # per-partition tables for a 2-head (128-row) o-tile
    cos64 = cosT[:, freq_idx].T              # [64, T]
    sgn = np.where(is_x1, -1.0, 1.0).astype(np.float32)
    sin64 = (sinT[:, freq_idx] * sgn[None, :]).T
    cossin = np.stack(
        [np.tile(cos64, (2, 1)), np.tile(sin64, (2, 1))], axis=1
    ).astype(np.float16)  # [128, 2, T]

    qw2 = np.concatenate([q_norm_w[perm64], q_norm_w[perm64]])  # [128]
    kw2 = np.concatenate([k_norm_w[perm64], k_norm_w[perm64]])
    qkw = np.stack(
        [
            1.0 / (D * qw2 * qw2),
            1.0 / (D * kw2 * kw2),
            EPS / (qw2 * qw2),
            EPS / (kw2 * kw2),
        ],
        axis=1,
    ).astype(np.float32)  # [128, 4] = scale q/k, bias q/k

    tables = np.zeros((128, 256), dtype=np.float16)
    tables[0:64, 0:64] = 1.0
    tables[64:128, 64:128] = 1.0
    # causal mask for a diagonal 128x128 block: valid iff s <= t
    s_i = np.arange(128)[:, None]
    t_i = np.arange(128)[None, :]
    tables[:, 128:256] = (s_i <= t_i).astype(np.float16)

    woT = np.ascontiguousarray(wo.T).astype(np.float16)  # [hd, e]

    xT_b = [np.ascontiguousarray(x[b].T).astype(np.float16) for b in range(B)]

    # per-head row interleave for the q and k blocks of wqkv
    perm256 = np.concatenate([64 * hh + perm64 for hh in range(HL)])

    in_maps = []
    for c in range(N_CORES):
        b, g = c // 4, c % 4
        rq = slice(256 * g, 256 * g + 256)
        wsel = np.concatenate(
            [wqkv[rq][perm256], wqkv[C:][rq][perm256], wqkv[2 * C :][rq]], axis=0
        )  # [768, C]
        wqkvT = np.ascontiguousarray(wsel.T).astype(np.float16)
        in_maps.append(
            {
                "xT": xT_b[b],
                "wqkvT": wqkvT,
                "woT": woT,
                "cossin": cossin,
                "qkw": qkw,
                "tables": tables,
            }
        )
    return in_maps


def get_program():
    if "nc" not in _BUILD_CACHE:
        _BUILD_CACHE["nc"] = _build_program()
    return _BUILD_CACHE["nc"]


def kernel(x, wqkv, wo, q_norm_w, k_norm_w):
    nc = get_program()
    in_maps = _host_inputs(x, wqkv, wo, q_norm_w, k_norm_w)
    res = run_bass_kernel_spmd(nc, in_maps, core_ids=list(range(N_CORES)))
    full = np.empty((B, T, C), dtype=np.float32)
    for c in range(N_CORES):
        o = res.results[c]["out"]  # [B, TS, C] f16
        full[:, TS * c : TS * (c + 1), :] = np.asarray(o, dtype=np.float32)
    return full


# revision 40
# speedup vs baseline: 1.0988x; 1.0032x over previous
"""Distributed causal self-attention kernel for 8 Trainium2 NeuronCores.

Problem: B=2, T=2048, C=1024, H=16 heads, D=64 head dim.
    qkv = x @ wqkv.T; q,k = rmsnorm(q|k)*w; rope; causal attention; out @ wo.T

Sharding: core c handles batch b = c//4 and head group g = c%4 (4 heads).
Per core:
  - QKV projection for its (b, heads) in transposed [o, t] layout (q, k)
    plus natural [s, d] layout for v.  q and k are both packed 2 heads per
    128-partition tile; the per-head QK matmul contracts over K=64 via
    base-partition-64 operand slices (no zero padding).
  - RMSNorm across d via a block-diagonal ones-matmul that sums and
    broadcasts per head in one shot.  rsqrt is computed as exp(-0.5*ln(y))
    so every scalar-engine activation (Exp/Ln/Copy) lives in ONE hardware
    table set and no table reloads occur.
  - RoPE via a single cross-partition stream_shuffle: the head dim is laid
    out host-side so each rope partner pair sits 16 partitions apart inside
    one 32-partition quadrant; the sign-folded sin table makes
    out = x*cos + shuffle(x)*sin exact.
  - Causal attention per head in S^T = [s, t] layout.  ||q||=||k||=sqrt(D)
    after rmsnorm, so scores are bounded and softmax needs no max
    subtraction.  exp runs on psum score pairs; only true diagonal
    128x128 blocks are masked, by one triangular table on the (otherwise
    idle) gpsimd engine.
  - AV runs TRANSPOSED: for each 128-wide t-block, matmul(lhsT=es_block,
    rhs=v_chunk) accumulates [t, d+1] in psum with N=65 per matmul --
    about half the tensor-engine streaming cost of the [d, t] form.  The
    ones column of V gives the softmax denominator; the division is a
    per-partition reciprocal + broadcast multiply during psum evacuation.
  - ONE AllToAll (8 cores) swaps head-shards for T-shards in [t, h, d]
    layout; each core then DMA-TRANSPOSES (xbar dma transpose, ~14ns/tile)
    its received blocks into [hd, t] tiles and runs the wo projection with
    no further reduction.

Matmul operands are float16 (fp32 PSUM accumulation).
"""

import numpy as np

import concourse.bass as bass
import concourse.mybir as mybir
import concourse.tile as tile
from concourse import bacc
from concourse.bass_utils import run_bass_kernel_spmd

N_CORES = 8
B, T, C = 2, 2048, 1024
H, D = 16, 64          # global heads, head dim
HL = 4                 # heads per core
ROPE_THETA = 10000.0
EPS = 1e-6
KO = C // 128          # contraction chunks for C
NT = T // 512          # 512-wide t-chunks
NS = T // 128          # 128-wide s-chunks
TS = T // N_CORES      # t-rows owned per core after AllToAll (256)

F16 = mybir.dt.float16
F32 = mybir.dt.float32

_BUILD_CACHE = {}
SKIP_COLLECTIVE = False  # drop the AllToAll (for single-core TimelineSim)
N_COLLECTIVES = 1

# rope partner shuffle: swap 16-partition halves within each 32-partition
# quadrant (see _host_inputs for the matching weight-row interleave)
SWAP16 = [(i + 16) % 32 for i in range(32)]


def _build_program():
    nc = bacc.Bacc(
        "TRN2",
        target_bir_lowering=False,
        debug=False,
        enable_asserts=False,
        num_devices=N_CORES,
    )
    xT = nc.dram_tensor("xT", [C, T], F16, kind="ExternalInput").ap()
    wqkvT = nc.dram_tensor("wqkvT", [C, 3 * HL * D], F16, kind="ExternalInput").ap()
    woT = nc.dram_tensor("woT", [H * D, C], F16, kind="ExternalInput").ap()
    # rope tables: [:, 0] = cos, [:, 1] = sign-folded sin
    cossin = nc.dram_tensor("cossin", [128, 2, T], F16, kind="ExternalInput").ap()
    # ln scale/bias with the norm weight folded in ([:, 0:2] = scale q/k,
    # [:, 2:4] = bias q/k):
    # exp(-0.5*ln(sum*qkw_s + qkw_b)) == w * rsqrt(mean + eps) for w > 0
    qkw = nc.dram_tensor("qkw", [128, 4], F32, kind="ExternalInput").ap()
    # [:, 0:128] block-diag ones (rmsnorm sum), [:, 128:256] causal triangle
    tables = nc.dram_tensor("tables", [128, 256], F16, kind="ExternalInput").ap()
    out = nc.dram_tensor("out", [B, TS, C], F16, kind="ExternalOutput").ap()

    with tile.TileContext(nc) as tc:
        _emit(tc, xT, wqkvT, woT, cossin, qkw, tables, out)
    nc.compile()
    return nc


def _emit(tc, xT, wqkvT, woT, cossin, qkw, tables, out):
    nc = tc.nc
    Exp = mybir.ActivationFunctionType.Exp
    Ln = mybir.ActivationFunctionType.Ln

    import contextlib

    with contextlib.ExitStack() as ctx:
        const = ctx.enter_context(tc.tile_pool(name="const", bufs=1))
        work = ctx.enter_context(tc.tile_pool(name="work", bufs=3))
        work3 = ctx.enter_context(tc.tile_pool(name="work3", bufs=4))
        espool = ctx.enter_context(tc.tile_pool(name="espool", bufs=4))
        ps512 = ctx.enter_context(tc.tile_pool(name="ps512", bufs=2, space="PSUM"))
        pss2 = ctx.enter_context(tc.tile_pool(name="pss2", bufs=2, space="PSUM"))
        po4p = ctx.enter_context(tc.tile_pool(name="po4p", bufs=2, space="PSUM"))
        dram = ctx.enter_context(tc.tile_pool(name="dram", bufs=1, space="DRAM"))

        # ---- persistent SBUF tiles ----
        xT_sb = const.tile([128, KO, T], F16, tag="xT_sb")
        wqkvT_sb = const.tile([128, KO, 3 * HL * D], F16, tag="wqkvT_sb")
        woT_sb = const.tile([128, KO, C], F16, tag="woT_sb")
        cossin_sb = const.tile([128, 2, T], F16, tag="cossin_sb")
        qkw_sb = const.tile([128, 4], F32, tag="qkw_sb")
        tables_sb = const.tile([128, 256], F16, tag="tables_sb")
        cos2_sb = cossin_sb[:, 0, :]
        sin2_sb = cossin_sb[:, 1, :]
        ones_sb = tables_sb[:, 0:128]
        mtri_sb = tables_sb[:, 128:256]
        qT = [
            const.tile([128, T], F16, tag=f"qT{p}", name=f"qT{p}") for p in range(2)
        ]
        kT = [
            const.tile([128, T], F16, tag=f"kT{p}", name=f"kT{p}") for p in range(2)
        ]
        vt = const.tile([128, NS, HL * (D + 1)], F16, tag="vt")
        # one aT tile per (batch, t-slot) so each out-projection group only
        # depends on its own 4 transposes, not all 16
        aTq = [
            const.tile([128, KO, 128], F16, tag=f"aTq{i}", name=f"aTq{i}")
            for i in range(4)
        ]

        # collective buffer in [t, h, d] layout:
        # (dst core, t-slot of 128, 128 t, 4 heads, 64 d)
        cc = dram.tile([N_CORES, 2, 128, HL, D], F16, tag="cc")
        cc_o = dram.tile([N_CORES, 2, 128, HL, D], F16, tag="cc_o")

        # ---- input DMAs (batched to keep HWDGE serialization low; the
        # first 512 t-columns of x and the qkv weights come first, small
        # tables next, then the rest of x; woT last -- it is only needed
        # by the output projection) ----
        xT_r = xT.rearrange("(ko p) t -> p ko t", p=128)
        wq_r = wqkvT.rearrange("(ko p) o -> p ko o", p=128)
        # first 512 t-columns of x and the weights in 2-ko chunks so the
        # in-order PE can start the first projection accumulation while the
        # rest streams in
        for koq in range(4):
            ks = bass.ts(koq, 2)
            nc.sync.dma_start(xT_sb[:, ks, 0:512], xT_r[:, ks, 0:512])
            nc.sync.dma_start(wqkvT_sb[:, ks, :], wq_r[:, ks, :])
        nc.sync.dma_start(qkw_sb[:], qkw[:])
        nc.sync.dma_start(tables_sb[:], tables[:])
        nc.sync.dma_start(cossin_sb[:, :, 0:512], cossin[:, :, 0:512])
        for koh in range(2):
            ks = bass.ts(koh, 4)
            nc.sync.dma_start(xT_sb[:, ks, 512:1024], xT_r[:, ks, 512:1024])
        nc.sync.dma_start(cossin_sb[:, :, 512:T], cossin[:, :, 512:T])
        for koh in range(2):
            ks = bass.ts(koh, 4)
            nc.sync.dma_start(xT_sb[:, ks, 1024:T], xT_r[:, ks, 1024:T])
        nc.sync.dma_start(woT_sb[:], woT.rearrange("(ko p) e -> p ko e", p=128))

        # ones column in the V tiles (drives the softmax denominator)
        nc.vector.memset(
            vt[:].rearrange("p s (h c) -> p s h c", c=D + 1)[:, :, :, D : D + 1], 1.0
        )

        # Load activation-table set 6 (natural_log_exp_and_others) ONCE up
        # front: it contains every function this kernel uses (Exp, Ln, Copy,
        # Square), so the Bacc fixpoint pass never needs to insert another
        # (it would otherwise thrash between the ln-only and exp-only sets).
        nc.scalar.add_instruction(
            mybir.InstLoadActFuncSet(
                name=nc.get_next_instruction_name(),
                ins=[],
                outs=[],
                act_func_set_id=6,
            )
        )

        # ---- thunk machinery: the PE executes strictly in order, so small
        # PE work units (projection ko-chunks, the norm ones-matmul) are
        # queued and popped between attention pairs.  This fills the PE
        # while the scalar engine's exp gates each pair's AV matmuls, and
        # it naturally delays each norm matmul until its DVE-side sum of
        # squares is long done. ----
        from collections import deque

        # entries are (deadline_key, thunk); deadline_key = (ti, si) means
        # the thunk MUST have run before attention chunk `ti` emits the pair
        # containing s-chunk `si` (si = -1: before chunk ti's first pair).
        # Pops are allowed any time; flush_until forces overdue thunks.
        filler = deque()

        def pop_filler(n=1):
            for _ in range(n):
                if filler:
                    filler.popleft()[1]()

        def flush_until(key):
            while filler and filler[0][0] <= key:
                filler.popleft()[1]()

        def flush_filler():
            while filler:
                filler.popleft()[1]()

        def qk_thunks(ot, ti):
            """q/k projection + rmsnorm + rope for one head-pair o-tile and
            one 512-wide t-chunk, split into 3 PE-granular thunks."""
            is_q = ot < 2
            pair = ot % 2
            wcol = 0 if is_q else 1
            dest = qT[pair] if is_q else kT[pair]
            tsl = bass.ts(ti, 512)
            st = {}

            def mk_proj(k0):
                def tp():
                    if k0 == 0:
                        st["ps"] = ps512.tile([128, 512], F32, tag="ps512", name="ps")
                    ps = st["ps"]
                    for ko in range(k0, k0 + 2):
                        nc.tensor.matmul(
                            ps[:],
                            lhsT=wqkvT_sb[:, ko, bass.ts(ot, 128)],
                            rhs=xT_sb[:, ko, tsl],
                            start=(ko == 0),
                            stop=(ko == KO - 1),
                        )
                    if k0 + 2 == KO:
                        # Evacuate psum immediately (frees the bank for the
                        # next accumulation), run the chain from the f16 copy.
                        rawe = work3.tile([128, 512], F16, tag="rawe", name="rawe")
                        nc.vector.tensor_copy(rawe[:], ps[:])
                        # rope partner shuffle does not depend on the norm
                        # coefficient (cb is constant across partner rows, so
                        # shuffle(raw*cb) == shuffle(raw)*cb and cb folds
                        # into the cos/sin coefficient tiles in t3).
                        rsw = work3.tile([128, 512], F16, tag="rsw", name="rsw")
                        nc.vector.stream_shuffle(rsw[:], rawe[:], SWAP16)
                        sq = work3.tile([128, 512], F16, tag="sq", name="sq")
                        nc.vector.tensor_mul(sq[:], rawe[:], rawe[:])
                        st["rawe"], st["rsw"], st["sq"] = rawe, rsw, sq

                return tp

            def t3():
                # Per-head sum of squares, replicated across the head's 64
                # partitions by the block-diagonal ones lhsT in one matmul.
                # The norm weight is folded into the ln's per-partition
                # scale/bias; rsqrt(y) = exp(-0.5*ln(y)) keeps the scalar
                # engine inside ONE activation-table set (Exp/Ln/Copy).
                ms = ps512.tile([128, 512], F32, tag="ps512", name="ms")
                nc.tensor.matmul(
                    ms[:], lhsT=ones_sb[:], rhs=st["sq"][:], start=True, stop=True
                )
                lncb = work.tile([128, 512], F32, tag="lncb", name="lncb")
                nc.scalar.activation(
                    lncb[:],
                    ms[:],
                    Ln,
                    bias=qkw_sb[:, 2 + wcol : 3 + wcol],
                    scale=qkw_sb[:, wcol : wcol + 1],
                )
                cb = work.tile([128, 512], F16, tag="cb", name="cb")
                nc.scalar.activation(cb[:], lncb[:], Exp, scale=-0.5)
                # dest = rawe*(cb*cos) + shuffle(rawe)*(cb*sin)
                cbc = work.tile([128, 512], F16, tag="cbc", name="cbc")
                nc.vector.tensor_mul(cbc[:], cb[:], cos2_sb[:, tsl])
                cbs = work.tile([128, 512], F16, tag="cbs", name="cbs")
                nc.vector.tensor_mul(cbs[:], cb[:], sin2_sb[:, tsl])
                rot = work.tile([128, 512], F16, tag="rot", name="rot")
                nc.vector.tensor_mul(rot[:], st["rsw"][:], cbs[:])
                cosm = work.tile([128, 512], F16, tag="cosm", name="cosm")
                nc.vector.tensor_mul(cosm[:], st["rawe"][:], cbc[:])
                nc.vector.tensor_add(dest[:, tsl], cosm[:], rot[:])

            return [mk_proj(0), mk_proj(2), mk_proj(4), mk_proj(6), t3]

        def v_thunks(st_):
            vt_heads = vt[:].rearrange("p s (h c) -> p s h c", c=D + 1)
            state = {}

            def mk_v(k0):
                def tv():
                    if k0 == 0:
                        state["psv"] = ps512.tile(
                            [128, 512], F32, tag="ps512", name="psv"
                        )
                    psv = state["psv"]
                    for ko in range(k0, k0 + 4):
                        nc.tensor.matmul(
                            psv[:, 0 : HL * D],
                            lhsT=xT_sb[:, ko, bass.ts(st_, 128)],
                            rhs=wqkvT_sb[:, ko, 2 * HL * D : 3 * HL * D],
                            start=(ko == 0),
                            stop=(ko == KO - 1),
                        )
                    if k0 + 4 == KO:
                        nc.vector.tensor_copy(
                            vt_heads[:, st_, :, 0:D],
                            psv[:, 0 : HL * D].rearrange("p (h c) -> p h c", c=D),
                        )

                return tv

            return [mk_v(0), mk_v(4)]

        def emit_attn(h, ti):
            """attention for one head and one 512-wide t-chunk.  Scores come
            in s-chunk pairs sharing a [128, 2, 512] psum tile (one exp per
            pair); AV accumulates transposed [t, d+1] tiles per 128-wide
            t-block with N=65 matmuls.  The pair loop is software-pipelined:
            pair p's AV runs after pair p+1's QK (plus one filler thunk), so
            the exp latency hides behind real PE work."""
            pair = h // 2
            hrow = bass.ds(64 * (h % 2), 64)
            po4 = po4p.tile([128, 4, D + 1], F32, tag="po4", name="po4")
            n_si = 4 * ti + 4
            av_state = {"first": True}

            def po4_mm(es, u, tj, cut, si, stop):
                nc.tensor.matmul(
                    po4[:, tj, :],
                    lhsT=es[:, u, bass.ds(128 * tj - cut, 128)],
                    rhs=vt[:, si, bass.ds(h * (D + 1), D + 1)],
                    start=av_state["first"],
                    stop=stop,
                )
                av_state["first"] = False

            def emit_av(p):
                # transposed AV: accumulate [t, d+1] per 128-wide t-block.
                # The whole po4 bank is ONE psum accumulation group (psum
                # group start/stop is bank-granular); per-element
                # has_written bits make the first write to each t-block an
                # overwrite and later ones accumulates.  Blocks that need no
                # causal mask go first so only the diagonal blocks wait on
                # the gpsimd mask ops.
                (es, j, cut) = p
                si0 = 4 * ti + j
                for u in range(2):
                    si = si0 + u
                    for tj in range(max(0, j + u + 1), 4):
                        po4_mm(es, u, tj, cut, si, False)
                # mask ONLY the true diagonal 128x128 blocks, on gpsimd
                for u in range(2):
                    tj = j + u
                    if tj >= 0:
                        blk = es[:, u, bass.ds(128 * tj - cut, 128)]
                        nc.gpsimd.tensor_mul(blk, blk, mtri_sb[:])
                for u in range(2):
                    tj = j + u
                    if tj >= 0:
                        po4_mm(es, u, tj, cut, 4 * ti + j + u, tj == 3)

            prev = None
            for si0 in range(0, n_si, 2):
                flush_until((ti, h, si0 + 1))
                j = si0 - 4 * ti
                # the last (most-diagonal) pair only attends within the
                # upper half of the t-chunk -- the lower half is fully
                # masked zeros, so compute it at half width
                last = j == 2
                tw = 256 if last else 512
                cut = 512 - tw           # es col 0 == chunk-local t = cut
                toff = 512 * ti + cut
                ps2 = pss2.tile([128, 2, 512], F32, tag="pss2", name="ps2")
                for u in range(2):
                    nc.tensor.matmul(
                        ps2[:, u, 0:tw],
                        lhsT=kT[pair][hrow, bass.ts(si0 + u, 128)],
                        rhs=qT[pair][hrow, bass.ds(toff, tw)],
                        start=True,
                        stop=True,
                    )
                es = espool.tile([128, 2, 512], F16, tag="es", name="es")
                nc.scalar.activation(
                    es[:, :, 0:tw], ps2[:, :, 0:tw], Exp, scale=1.0 / np.sqrt(D)
                )
                if prev is not None:
                    pop_filler()
                    emit_av(prev)
                prev = (es, j, cut)
            pop_filler()
            emit_av(prev)
            # softmax division on evacuation: per-partition reciprocal of
            # the ones-column, broadcast along d
            rec4 = work.tile([128, 4], F32, tag="rec4", name="rec4")
            nc.vector.reciprocal(
                rec4[:], po4[:, :, D : D + 1].rearrange("p a b -> p (a b)")
            )
            ob = work.tile([128, 4, D], F16, tag="ob", name="ob")
            nc.vector.tensor_mul(
                ob[:], po4[:, :, 0:D], rec4[:].broadcast_to([128, 4, D])
            )
            # one DMA: t-block tj goes to dst core (4*ti+tj)//2, slot
            # (4*ti+tj)%2 -- contiguous (dst, slot) blocks in cc.  Issued
            # via the gpsimd SWDGE queue: the SP queue's DMA counting
            # semaphore then never chains the tail transposes behind these
            # writes.
            ccr = cc[:].rearrange("dst slot t h d -> t (dst slot) h d")
            nc.sync.dma_start(ccr[:, bass.ds(4 * ti, 4), h, :], ob[:])

        def emit_a2a():
            if SKIP_COLLECTIVE:
                return
            nc.gpsimd.collective_compute(
                "AllToAll",
                mybir.AluOpType.bypass,
                replica_groups=[list(range(N_CORES))],
                ins=[cc.opt()],
                outs=[cc_o.opt()],
            )

        def emit_tail():
            # aT column b*KO+ko holds hd rows [128*ko, 128*ko+128) of batch
            # b = global heads {2*ko, 2*ko+1}: source core 4*b + g covers
            # columns b*KO + {2g, 2g+1}.  One xbar dma transpose per
            # (source core, t-slot) turns the received [128 t, 4 h, 64 d]
            # block into both aT column tiles at once; the out projection
            # for that (b, t-slot) follows immediately so PE work overlaps
            # the remaining transposes.  In SKIP_COLLECTIVE timing mode the
            # transposes read cc itself, which models the true
            # after-all-attention ordering without the collective.
            src_buf = cc if SKIP_COLLECTIVE else cc_o
            for b in range(B):
                for slot in range(2):
                    aT = aTq[2 * b + slot]
                    for g in range(4):
                        nc.sync.dma_start_transpose(
                            aT[:, bass.ds(2 * g, 2), :],
                            src_buf[4 * b + g, slot, :, :, :],
                        )
                    tt = slot
                    for ec in range(C // 512):
                        pout = ps512.tile([128, 512], F32, tag="ps512", name="pout")
                        for ko in range(KO):
                            nc.tensor.matmul(
                                pout[:],
                                lhsT=aT[:, ko, :],
                                rhs=woT_sb[:, ko, bass.ts(ec, 512)],
                                start=(ko == 0),
                                stop=(ko == KO - 1),
                            )
                        ob = work.tile([128, 512], F16, tag="obp", name="obp")
                        nc.vector.tensor_copy(ob[:], pout[:])
                        nc.sync.dma_start(
                            out[b, bass.ts(tt, 128), bass.ts(ec, 512)], ob[:]
                        )

        # ---- emission order: chunk-0 projections up front (norm matmuls
        # staggered one slot behind their projections), then per 512-wide
        # t-chunk the NEXT chunk's projection thunks are queued and popped
        # between attention pairs; then ONE AllToAll, the dma-transposed
        # aT loads, and the output projection ----
        ot_order = (0, 2, 1, 3)
        pre = [qk_thunks(ot, 0) for ot in ot_order]
        vpre = [v_thunks(st) for st in range(4)]
        for i in range(4):
            for tp in pre[i][:4]:
                tp()
            if i > 0:
                pre[i - 1][4]()
            vpre[i][0]()
            vpre[i][1]()
        pre[3][4]()
        for ti in range(NT):
            # queue the next chunk's projection thunks with deadlines;
            # attention pairs pop one between each score/AV step so the PE
            # never idles while the scalar engine's exp stream gates the
            # AVs.  Q tiles are due at the next chunk's first pair, but K
            # tiles and V chunks are only due at the pair that first reads
            # them -- so they spill INTO the next chunk's attention and keep
            # feeding the PE during the exp-bound final chunk.
            if ti + 1 < NT:
                t1 = ti + 1
                for h in range(HL):
                    filler.extend(((t1, 0, -1), t) for t in qk_thunks(ot_order[h], t1))
                    filler.extend(((t1, 0, -1), t) for t in v_thunks(4 * t1 + h))
            for h in range(HL):
                emit_attn(h, ti)
            flush_filler()
        emit_a2a()
        emit_tail()


def _host_inputs(x, wqkv, wo, q_norm_w, k_norm_w):
    """Build the per-core input maps (all host-side prep is layout/dtype only)."""
    x = np.asarray(x, dtype=np.float32)
    wqkv = np.asarray(wqkv, dtype=np.float32)
    wo = np.asarray(wo, dtype=np.float32)
    q_norm_w = np.asarray(q_norm_w, dtype=np.float32)
    k_norm_w = np.asarray(k_norm_w, dtype=np.float32)

    # head-dim interleave: rope partner pairs (x1_i, x2_i) sit 16 partitions
    # apart within one 32-partition quadrant, so ONE stream_shuffle (swap
    # 16-halves per quadrant) aligns every partner.  perm64[new] = old.
    perm64 = np.concatenate(
        [np.arange(0, 16), np.arange(32, 48), np.arange(16, 32), np.arange(48, 64)]
    )
    # rope frequency index and partner-sign per (new) position
    j = np.arange(64)
    q32, r32 = j // 32, j % 32
    freq_idx = 16 * q32 + (r32 % 16)          # pair index i in [0, 32)
    is_x1 = r32 < 16                          # rows holding x1_i

    inv_freq = (
        1.0 / (ROPE_THETA ** (np.arange(0, D, 2, dtype=np.float32) / D))
    ).astype(np.float32)
    freqs = np.arange(T, dtype=np.float32)[:, None] * inv_freq[None, :]  # [T, 32]
    cosT = np.cos(freqs).astype(np.float32)  # [T, 32]
    sinT = np.sin(freqs).astype(np.float32)
    # per-partition tables for a 2-head (128-row) o-tile
    cos64 = cosT[:, freq_idx].T              # [64, T]
    sgn = np.where(is_x1, -1.0, 1.0).astype(np.float32)
    sin64 = (sinT[:, freq_idx] * sgn[None, :]).T
    cossin = np.stack(
        [np.tile(cos64, (2, 1)), np.tile(sin64, (2, 1))], axis=1
    ).astype(np.float16)  # [128, 2, T]

    qw2 = np.concatenate([q_norm_w[perm64], q_norm_w[perm64]])  # [128]
    kw2 = np.concatenate([k_norm_w[perm64], k_norm_w[perm64]])
    qkw = np.stack(
        [
            1.0 / (D * qw2 * qw2),
            1.0 / (D * kw2 * kw2),
            EPS / (qw2 * qw2),
            EPS / (kw2 * kw2),
        ],
        axis=1,
    ).astype(np.float32)  # [128, 4] = scale q/k, bias q/k

    tables = np.zeros((128, 256), dtype=np.float16)
    tables[0:64, 0:64] = 1.0
    tables[64:128, 64:128] = 1.0
    # causal mask for a diagonal 128x128 block: valid iff s <= t
    s_i = np.arange(128)[:, None]
    t_i = np.arange(128)[None, :]
    tables[:, 128:256] = (s_i <= t_i).astype(np.float16)

    woT = np.ascontiguousarray(wo.T).astype(np.float16)  # [hd, e]

    xT_b = [np.ascontiguousarray(x[b].T).astype(np.float16) for b in range(B)]

    # per-head row interleave for the q and k blocks of wqkv
    perm256 = np.concatenate([64 * hh + perm64 for hh in range(HL)])

    in_maps = []
    for c in range(N_CORES):
        b, g = c // 4, c % 4
        rq = slice(256 * g, 256 * g + 256)
        wsel = np.concatenate(
            [wqkv[rq][perm256], wqkv[C:][rq][perm256], wqkv[2 * C :][rq]], axis=0
        )  # [768, C]
        wqkvT = np.ascontiguousarray(wsel.T).astype(np.float16)
        in_maps.append(
            {
                "xT": xT_b[b],
                "wqkvT": wqkvT,
                "woT": woT,
                "cossin": cossin,
                "qkw": qkw,
                "tables": tables,
            }
        )
    return in_maps


def get_program():
    if "nc" not in _BUILD_CACHE:
        _BUILD_CACHE["nc"] = _build_program()
    return _BUILD_CACHE["nc"]


def kernel(x, wqkv, wo, q_norm_w, k_norm_w):
    nc = get_program()
    in_maps = _host_inputs(x, wqkv, wo, q_norm_w, k_norm_w)
    res = run_bass_kernel_spmd(nc, in_maps, core_ids=list(range(N_CORES)))
    full = np.empty((B, T, C), dtype=np.float32)
    for c in range(N_CORES):
        o = res.results[c]["out"]  # [B, TS, C] f16
        full[:, TS * c : TS * (c + 1), :] = np.asarray(o, dtype=np.float32)
    return full


# revision 53
# speedup vs baseline: 1.1334x; 1.0315x over previous
"""Distributed causal self-attention kernel for 8 Trainium2 NeuronCores.

Problem: B=2, T=2048, C=1024, H=16 heads, D=64 head dim.
    qkv = x @ wqkv.T; q,k = rmsnorm(q|k)*w; rope; causal attention; out @ wo.T

Sharding: core c handles batch b = c//4 and head group g = c%4 (4 heads).
Per core:
  - QKV projection for its (b, heads) in transposed [o, t] layout (q, k)
    plus natural [s, d] layout for v.  q and k are both packed 2 heads per
    128-partition tile; the per-head QK matmul contracts over K=64 via
    base-partition-64 operand slices (no zero padding).
  - RMSNorm across d via a block-diagonal ones-matmul that sums and
    broadcasts per head in one shot.  rsqrt is computed as exp(-0.5*ln(y))
    so every scalar-engine activation (Exp/Ln/Copy) lives in ONE hardware
    table set and no table reloads occur.
  - RoPE via a single cross-partition stream_shuffle: the head dim is laid
    out host-side so each rope partner pair sits 16 partitions apart inside
    one 32-partition quadrant; the sign-folded sin table makes
    out = x*cos + shuffle(x)*sin exact.
  - Causal attention per head in S^T = [s, t] layout.  ||q||=||k||=sqrt(D)
    after rmsnorm, so scores are bounded and softmax needs no max
    subtraction.  exp runs on psum score pairs; only true diagonal
    128x128 blocks are masked, by one triangular table on the (otherwise
    idle) gpsimd engine.
  - AV runs TRANSPOSED: for each 128-wide t-block, matmul(lhsT=es_block,
    rhs=v_chunk) accumulates [t, d+1] in psum with N=65 per matmul --
    about half the tensor-engine streaming cost of the [d, t] form.  The
    ones column of V gives the softmax denominator; the division is a
    per-partition reciprocal + broadcast multiply during psum evacuation.
  - ONE AllToAll (8 cores) swaps head-shards for T-shards in [t, h, d]
    layout; each core then DMA-TRANSPOSES (xbar dma transpose, ~14ns/tile)
    its received blocks into [hd, t] tiles and runs the wo projection with
    no further reduction.

Matmul operands are float16 (fp32 PSUM accumulation).
"""

import numpy as np

import concourse.bass as bass
import concourse.mybir as mybir
import concourse.tile as tile
from concourse import bacc
from concourse.bass_utils import run_bass_kernel_spmd

N_CORES = 8
B, T, C = 2, 2048, 1024
H, D = 16, 64          # global heads, head dim
HL = 4                 # heads per core
ROPE_THETA = 10000.0
EPS = 1e-6
KO = C // 128          # contraction chunks for C
NT = T // 512          # 512-wide t-chunks
NS = T // 128          # 128-wide s-chunks
TS = T // N_CORES      # t-rows owned per core after AllToAll (256)

F16 = mybir.dt.float16
F32 = mybir.dt.float32

_BUILD_CACHE = {}
SKIP_COLLECTIVE = False  # drop the AllToAll (for single-core TimelineSim)
N_COLLECTIVES = 1

# rope partner shuffle: swap 16-partition halves within each 32-partition
# quadrant (see _host_inputs for the matching weight-row interleave)
SWAP16 = [(i + 16) % 32 for i in range(32)]


def _build_program():
    nc = bacc.Bacc(
        "TRN2",
        target_bir_lowering=False,
        debug=False,
        enable_asserts=False,
        num_devices=N_CORES,
    )
    xT = nc.dram_tensor("xT", [C, T], F16, kind="ExternalInput").ap()
    wqkvT = nc.dram_tensor("wqkvT", [C, 3 * HL * D], F16, kind="ExternalInput").ap()
    woT = nc.dram_tensor("woT", [H * D, C], F16, kind="ExternalInput").ap()
    # rope tables: [:, 0] = cos, [:, 1] = sign-folded sin
    cossin = nc.dram_tensor("cossin", [128, 2, T], F16, kind="ExternalInput").ap()
    # ln scale/bias with the norm weight folded in ([:, 0:2] = scale q/k,
    # [:, 2:4] = bias q/k):
    # exp(-0.5*ln(sum*qkw_s + qkw_b)) == w * rsqrt(mean + eps) for w > 0
    qkw = nc.dram_tensor("qkw", [128, 4], F32, kind="ExternalInput").ap()
    # [:, 0:128] block-diag ones (rmsnorm sum), [:, 128:256] causal triangle
    tables = nc.dram_tensor("tables", [128, 256], F16, kind="ExternalInput").ap()
    out = nc.dram_tensor("out", [B, TS, C], F16, kind="ExternalOutput").ap()

    with tile.TileContext(nc) as tc:
        _emit(tc, xT, wqkvT, woT, cossin, qkw, tables, out)
    nc.compile()
    return nc


def _emit(tc, xT, wqkvT, woT, cossin, qkw, tables, out):
    nc = tc.nc
    Exp = mybir.ActivationFunctionType.Exp
    Ln = mybir.ActivationFunctionType.Ln

    import contextlib

    with contextlib.ExitStack() as ctx:
        const = ctx.enter_context(tc.tile_pool(name="const", bufs=1))
        work = ctx.enter_context(tc.tile_pool(name="work", bufs=4))
        work3 = ctx.enter_context(tc.tile_pool(name="work3", bufs=6))
        espool = ctx.enter_context(tc.tile_pool(name="espool", bufs=6))
        ps512 = ctx.enter_context(tc.tile_pool(name="ps512", bufs=2, space="PSUM"))
        pss2 = ctx.enter_context(tc.tile_pool(name="pss2", bufs=2, space="PSUM"))
        po4p = ctx.enter_context(tc.tile_pool(name="po4p", bufs=2, space="PSUM"))
        dram = ctx.enter_context(tc.tile_pool(name="dram", bufs=1, space="DRAM"))

        # ---- persistent SBUF tiles ----
        xT_sb = const.tile([128, KO, T], F16, tag="xT_sb")
        wqkvT_sb = const.tile([128, KO, 3 * HL * D], F16, tag="wqkvT_sb")
        woT_sb = const.tile([128, KO, C], F16, tag="woT_sb")
        cossin_sb = const.tile([128, 2, T], F16, tag="cossin_sb")
        qkw_sb = const.tile([128, 4], F32, tag="qkw_sb")
        tables_sb = const.tile([128, 256], F16, tag="tables_sb")
        cos2_sb = cossin_sb[:, 0, :]
        sin2_sb = cossin_sb[:, 1, :]
        ones_sb = tables_sb[:, 0:128]
        mtri_sb = tables_sb[:, 128:256]
        qT = [
            const.tile([128, T], F16, tag=f"qT{p}", name=f"qT{p}") for p in range(2)
        ]
        kT = [
            const.tile([128, T], F16, tag=f"kT{p}", name=f"kT{p}") for p in range(2)
        ]
        vt = const.tile([128, NS, HL * (D + 1)], F16, tag="vt")
        # one aT tile per (batch, t-slot) so each out-projection group only
        # depends on its own 4 transposes, not all 16
        aTq = [
            const.tile([128, KO, 128], F16, tag=f"aTq{i}", name=f"aTq{i}")
            for i in range(4)
        ]

        # collective buffer in [t, h, d] layout:
        # (dst core, t-slot of 128, 128 t, 4 heads, 64 d)
        cc = dram.tile([N_CORES, 2, 128, HL, D], F16, tag="cc")
        cc_o = dram.tile([N_CORES, 2, 128, HL, D], F16, tag="cc_o")

        # ---- input DMAs (batched to keep HWDGE serialization low; the
        # first 512 t-columns of x and the qkv weights come first, small
        # tables next, then the rest of x; woT last -- it is only needed
        # by the output projection) ----
        xT_r = xT.rearrange("(ko p) t -> p ko t", p=128)
        wq_r = wqkvT.rearrange("(ko p) o -> p ko o", p=128)
        # first 512 t-columns of x and the weights in 2-ko chunks so the
        # in-order PE can start the first projection accumulation while the
        # rest streams in
        for koq in range(4):
            ks = bass.ts(koq, 2)
            nc.sync.dma_start(xT_sb[:, ks, 0:512], xT_r[:, ks, 0:512])
            nc.sync.dma_start(wqkvT_sb[:, ks, :], wq_r[:, ks, :])
        nc.sync.dma_start(qkw_sb[:], qkw[:])
        nc.sync.dma_start(tables_sb[:], tables[:])
        nc.sync.dma_start(cossin_sb[:, :, 0:512], cossin[:, :, 0:512])
        for koh in range(2):
            ks = bass.ts(koh, 4)
            nc.sync.dma_start(xT_sb[:, ks, 512:1024], xT_r[:, ks, 512:1024])
        nc.sync.dma_start(cossin_sb[:, :, 512:T], cossin[:, :, 512:T])
        for koh in range(2):
            ks = bass.ts(koh, 4)
            nc.sync.dma_start(xT_sb[:, ks, 1024:T], xT_r[:, ks, 1024:T])
        nc.sync.dma_start(woT_sb[:], woT.rearrange("(ko p) e -> p ko e", p=128))

        # ones column in the V tiles (drives the softmax denominator)
        nc.vector.memset(
            vt[:].rearrange("p s (h c) -> p s h c", c=D + 1)[:, :, :, D : D + 1], 1.0
        )

        # Load activation-table set 6 (natural_log_exp_and_others) ONCE up
        # front: it contains every function this kernel uses (Exp, Ln, Copy,
        # Square), so the Bacc fixpoint pass never needs to insert another
        # (it would otherwise thrash between the ln-only and exp-only sets).
        nc.scalar.add_instruction(
            mybir.InstLoadActFuncSet(
                name=nc.get_next_instruction_name(),
                ins=[],
                outs=[],
                act_func_set_id=6,
            )
        )

        # ---- thunk machinery: the PE executes strictly in order, so small
        # PE work units (projection ko-chunks, the norm ones-matmul) are
        # queued and popped between attention pairs.  This fills the PE
        # while the scalar engine's exp gates each pair's AV matmuls, and
        # it naturally delays each norm matmul until its DVE-side sum of
        # squares is long done. ----
        from collections import deque

        # entries are (deadline_key, thunk); deadline_key = (ti, si) means
        # the thunk MUST have run before attention chunk `ti` emits the pair
        # containing s-chunk `si` (si = -1: before chunk ti's first pair).
        # Pops are allowed any time; flush_until forces overdue thunks.
        filler = deque()

        def pop_filler(n=1):
            for _ in range(n):
                if filler:
                    filler.popleft()[1]()

        def flush_until(key):
            while filler and filler[0][0] <= key:
                filler.popleft()[1]()

        def flush_filler():
            while filler:
                filler.popleft()[1]()

        def qk_thunks(ot, ti):
            """q/k projection + rmsnorm + rope for one head-pair o-tile and
            one 512-wide t-chunk, split into 3 PE-granular thunks."""
            is_q = ot < 2
            pair = ot % 2
            wcol = 0 if is_q else 1
            dest = qT[pair] if is_q else kT[pair]
            tsl = bass.ts(ti, 512)
            st = {}

            def mk_proj(k0):
                def tp():
                    if k0 == 0:
                        st["ps"] = ps512.tile([128, 512], F32, tag="ps512", name="ps")
                    ps = st["ps"]
                    for ko in range(k0, k0 + 2):
                        nc.tensor.matmul(
                            ps[:],
                            lhsT=wqkvT_sb[:, ko, bass.ts(ot, 128)],
                            rhs=xT_sb[:, ko, tsl],
                            start=(ko == 0),
                            stop=(ko == KO - 1),
                        )
                    if k0 + 2 == KO:
                        # Evacuate psum immediately (frees the bank for the
                        # next accumulation), run the chain from the f16 copy.
                        rawe = work3.tile([128, 512], F16, tag="rawe", name="rawe")
                        nc.vector.tensor_copy(rawe[:], ps[:])
                        # rope partner shuffle does not depend on the norm
                        # coefficient (cb is constant across partner rows, so
                        # shuffle(raw*cb) == shuffle(raw)*cb and cb folds
                        # into the cos/sin coefficient tiles in t3).
                        rsw = work3.tile([128, 512], F16, tag="rsw", name="rsw")
                        nc.vector.stream_shuffle(rsw[:], rawe[:], SWAP16)
                        sq = work3.tile([128, 512], F16, tag="sq", name="sq")
                        nc.vector.tensor_mul(sq[:], rawe[:], rawe[:])
                        st["rawe"], st["rsw"], st["sq"] = rawe, rsw, sq

                return tp

            def t3():
                # Per-head sum of squares, replicated across the head's 64
                # partitions by the block-diagonal ones lhsT in one matmul.
                # The norm weight is folded into the ln's per-partition
                # scale/bias; rsqrt(y) = exp(-0.5*ln(y)) keeps the scalar
                # engine inside ONE activation-table set (Exp/Ln/Copy).
                ms = ps512.tile([128, 512], F32, tag="ps512", name="ms")
                nc.tensor.matmul(
                    ms[:], lhsT=ones_sb[:], rhs=st["sq"][:], start=True, stop=True
                )
                lncb = work.tile([128, 512], F32, tag="lncb", name="lncb")
                nc.scalar.activation(
                    lncb[:],
                    ms[:],
                    Ln,
                    bias=qkw_sb[:, 2 + wcol : 3 + wcol],
                    scale=qkw_sb[:, wcol : wcol + 1],
                )
                cb = work.tile([128, 512], F16, tag="cb", name="cb")
                nc.scalar.activation(cb[:], lncb[:], Exp, scale=-0.5)
                # dest = rawe*(cb*cos) + shuffle(rawe)*(cb*sin)
                cbc = work.tile([128, 512], F16, tag="cbc", name="cbc")
                nc.vector.tensor_mul(cbc[:], cb[:], cos2_sb[:, tsl])
                cbs = work.tile([128, 512], F16, tag="cbs", name="cbs")
                nc.vector.tensor_mul(cbs[:], cb[:], sin2_sb[:, tsl])
                rot = work.tile([128, 512], F16, tag="rot", name="rot")
                nc.vector.tensor_mul(rot[:], st["rsw"][:], cbs[:])
                cosm = work.tile([128, 512], F16, tag="cosm", name="cosm")
                nc.vector.tensor_mul(cosm[:], st["rawe"][:], cbc[:])
                nc.vector.tensor_add(dest[:, tsl], cosm[:], rot[:])

            return [mk_proj(0), mk_proj(2), mk_proj(4), mk_proj(6), t3]

        def v_thunks(st_):
            vt_heads = vt[:].rearrange("p s (h c) -> p s h c", c=D + 1)
            state = {}

            def mk_v(k0):
                def tv():
                    if k0 == 0:
                        state["psv"] = ps512.tile(
                            [128, 512], F32, tag="ps512", name="psv"
                        )
                    psv = state["psv"]
                    for ko in range(k0, k0 + 4):
                        nc.tensor.matmul(
                            psv[:, 0 : HL * D],
                            lhsT=xT_sb[:, ko, bass.ts(st_, 128)],
                            rhs=wqkvT_sb[:, ko, 2 * HL * D : 3 * HL * D],
                            start=(ko == 0),
                            stop=(ko == KO - 1),
                        )
                    if k0 + 4 == KO:
                        nc.vector.tensor_copy(
                            vt_heads[:, st_, :, 0:D],
                            psv[:, 0 : HL * D].rearrange("p (h c) -> p h c", c=D),
                        )

                return tv

            return [mk_v(0), mk_v(4)]

        def emit_attn(h, ti):
            """attention for one head and one 512-wide t-chunk.  Scores come
            in s-chunk pairs sharing a [128, 2, 512] psum tile (one exp per
            pair); AV accumulates transposed [t, d+1] tiles per 128-wide
            t-block with N=65 matmuls.  The pair loop is software-pipelined:
            pair p's AV runs after pair p+1's QK (plus one filler thunk), so
            the exp latency hides behind real PE work."""
            pair = h // 2
            hrow = bass.ds(64 * (h % 2), 64)
            po4 = po4p.tile([128, 4, D + 1], F32, tag="po4", name="po4")
            n_si = 4 * ti + 4
            av_state = {"first": True}

            def po4_mm(es, u, tj, cut, si, stop):
                nc.tensor.matmul(
                    po4[:, tj, :],
                    lhsT=es[:, u, bass.ds(128 * tj - cut, 128)],
                    rhs=vt[:, si, bass.ds(h * (D + 1), D + 1)],
                    start=av_state["first"],
                    stop=stop,
                )
                av_state["first"] = False

            def emit_av(p):
                # transposed AV: accumulate [t, d+1] per 128-wide t-block.
                # The whole po4 bank is ONE psum accumulation group (psum
                # group start/stop is bank-granular); per-element
                # has_written bits make the first write to each t-block an
                # overwrite and later ones accumulates.  Blocks that need no
                # causal mask go first so only the diagonal blocks wait on
                # the gpsimd mask ops.
                (es, j, cut) = p
                si0 = 4 * ti + j
                for u in range(2):
                    si = si0 + u
                    for tj in range(max(0, j + u + 1), 4):
                        po4_mm(es, u, tj, cut, si, False)
                # mask ONLY the true diagonal 128x128 blocks, on gpsimd
                for u in range(2):
                    tj = j + u
                    if tj >= 0:
                        blk = es[:, u, bass.ds(128 * tj - cut, 128)]
                        nc.vector.tensor_mul(blk, blk, mtri_sb[:])
                for u in range(2):
                    tj = j + u
                    if tj >= 0:
                        po4_mm(es, u, tj, cut, 4 * ti + j + u, tj == 3)

            pend = []
            for si0 in range(0, n_si, 2):
                flush_until((ti, h, si0 + 1))
                j = si0 - 4 * ti
                # the last (most-diagonal) pair only attends within the
                # upper half of the t-chunk -- the lower half is fully
                # masked zeros, so compute it at half width
                last = j == 2
                tw = 256 if last else 512
                cut = 512 - tw           # es col 0 == chunk-local t = cut
                toff = 512 * ti + cut
                ps2 = pss2.tile([128, 2, 512], F32, tag="pss2", name="ps2")
                for u in range(2):
                    # columns left of this s-chunk's first valid t-block are
                    # fully masked -- skip computing them.  (The stale psum
                    # they leave behind is exp'd but its es block is never
                    # read by any AV matmul.  ti=0 keeps full width so the
                    # first use of each psum buffer is fully initialized.)
                    lo = max(0, 128 * (si0 + u) - toff) if ti > 0 else 0
                    nc.tensor.matmul(
                        ps2[:, u, lo:tw],
                        lhsT=kT[pair][hrow, bass.ts(si0 + u, 128)],
                        rhs=qT[pair][hrow, bass.ds(toff + lo, tw - lo)],
                        start=True,
                        stop=True,
                    )
                es = espool.tile([128, 2, 512], F16, tag="es", name="es")
                nc.scalar.activation(
                    es[:, :, 0:tw], ps2[:, :, 0:tw], Exp, scale=1.0 / np.sqrt(D)
                )
                pend.append((es, j, cut))
                if len(pend) > 1:
                    pop_filler()
                    emit_av(pend.pop(0))
            while pend:
                pop_filler()
                emit_av(pend.pop(0))
            # softmax division on evacuation: per-partition reciprocal of
            # the ones-column, broadcast along d
            rec4 = work.tile([128, 4], F32, tag="rec4", name="rec4")
            nc.vector.reciprocal(
                rec4[:], po4[:, :, D : D + 1].rearrange("p a b -> p (a b)")
            )
            ob = work.tile([128, 4, D], F16, tag="ob", name="ob")
            nc.vector.tensor_mul(
                ob[:], po4[:, :, 0:D], rec4[:].broadcast_to([128, 4, D])
            )
            # one DMA: t-block tj goes to dst core (4*ti+tj)//2, slot
            # (4*ti+tj)%2 -- contiguous (dst, slot) blocks in cc.  Issued
            # via the gpsimd SWDGE queue: the SP queue's DMA counting
            # semaphore then never chains the tail transposes behind these
            # writes.
            ccr = cc[:].rearrange("dst slot t h d -> t (dst slot) h d")
            nc.sync.dma_start(ccr[:, bass.ds(4 * ti, 4), h, :], ob[:])

        def emit_a2a():
            if SKIP_COLLECTIVE:
                return
            nc.gpsimd.collective_compute(
                "AllToAll",
                mybir.AluOpType.bypass,
                replica_groups=[list(range(N_CORES))],
                ins=[cc.opt()],
                outs=[cc_o.opt()],
            )

        def emit_tail():
            # aT column b*KO+ko holds hd rows [128*ko, 128*ko+128) of batch
            # b = global heads {2*ko, 2*ko+1}: source core 4*b + g covers
            # columns b*KO + {2g, 2g+1}.  One xbar dma transpose per
            # (source core, t-slot) turns the received [128 t, 4 h, 64 d]
            # block into both aT column tiles at once; the out projection
            # for that (b, t-slot) follows immediately so PE work overlaps
            # the remaining transposes.  In SKIP_COLLECTIVE timing mode the
            # transposes read cc itself, which models the true
            # after-all-attention ordering without the collective.
            src_buf = cc if SKIP_COLLECTIVE else cc_o
            for b in range(B):
                for slot in range(2):
                    aT = aTq[2 * b + slot]
                    for g in range(4):
                        nc.sync.dma_start_transpose(
                            aT[:, bass.ds(2 * g, 2), :],
                            src_buf[4 * b + g, slot, :, :, :],
                        )
                    tt = slot
                    for ec in range(C // 512):
                        pout = ps512.tile([128, 512], F32, tag="ps512", name="pout")
                        for ko in range(KO):
                            nc.tensor.matmul(
                                pout[:],
                                lhsT=aT[:, ko, :],
                                rhs=woT_sb[:, ko, bass.ts(ec, 512)],
                                start=(ko == 0),
                                stop=(ko == KO - 1),
                            )
                        ob = work.tile([128, 512], F16, tag="obp", name="obp")
                        nc.vector.tensor_copy(ob[:], pout[:])
                        nc.sync.dma_start(
                            out[b, bass.ts(tt, 128), bass.ts(ec, 512)], ob[:]
                        )

        # ---- emission order: chunk-0 projections up front (norm matmuls
        # staggered one slot behind their projections), then per 512-wide
        # t-chunk the NEXT chunk's projection thunks are queued and popped
        # between attention pairs; then ONE AllToAll, the dma-transposed
        # aT loads, and the output projection ----
        ot_order = (0, 2, 1, 3)
        pre = [qk_thunks(ot, 0) for ot in ot_order]
        vpre = [v_thunks(st) for st in range(4)]
        for i in range(4):
            for tp in pre[i][:4]:
                tp()
            if i > 0:
                pre[i - 1][4]()
            vpre[i][0]()
            vpre[i][1]()
        pre[3][4]()
        for ti in range(NT):
            # queue the next chunk's projection thunks with deadlines;
            # attention pairs pop one between each score/AV step so the PE
            # never idles while the scalar engine's exp stream gates the
            # AVs.  Q tiles are due at the next chunk's first pair, but K
            # tiles and V chunks are only due at the pair that first reads
            # them -- so they spill INTO the next chunk's attention and keep
            # feeding the PE during the exp-bound final chunk.
            if ti + 1 < NT:
                t1 = ti + 1
                for h in range(HL):
                    filler.extend(((t1, 0, -1), t) for t in qk_thunks(ot_order[h], t1))
                    filler.extend(((t1, 0, -1), t) for t in v_thunks(4 * t1 + h))
            for h in range(HL):
                emit_attn(h, ti)
            flush_filler()
        emit_a2a()
        emit_tail()


def _host_inputs(x, wqkv, wo, q_norm_w, k_norm_w):
    """Build the per-core input maps (all host-side prep is layout/dtype only)."""
    x = np.asarray(x, dtype=np.float32)
    wqkv = np.asarray(wqkv, dtype=np.float32)
    wo = np.asarray(wo, dtype=np.float32)
    q_norm_w = np.asarray(q_norm_w, dtype=np.float32)
    k_norm_w = np.asarray(k_norm_w, dtype=np.float32)

    # head-dim interleave: rope partner pairs (x1_i, x2_i) sit 16 partitions
    # apart within one 32-partition quadrant, so ONE stream_shuffle (swap
    # 16-halves per quadrant) aligns every partner.  perm64[new] = old.
    perm64 = np.concatenate(
        [np.arange(0, 16), np.arange(32, 48), np.arange(16, 32), np.arange(48, 64)]
    )
    # rope frequency index and partner-sign per (new) position
    j = np.arange(64)
    q32, r32 = j // 32, j % 32
    freq_idx = 16 * q32 + (r32 % 16)          # pair index i in [0, 32)
    is_x1 = r32 < 16                          # rows holding x1_i

    inv_freq = (
        1.0 / (ROPE_THETA ** (np.arange(0, D, 2, dtype=np.float32) / D))
    ).astype(np.float32)
    freqs = np.arange(T, dtype=np.float32)[:, None] * inv_freq[None, :]  # [T, 32]
    cosT = np.cos(freqs).astype(np.float32)  # [T, 32]
    sinT = np.sin(freqs).astype(np.float32)
    # per-partition tables for a 2-head (128-row) o-tile
    cos64 = cosT[:, freq_idx].T              # [64, T]
    sgn = np.where(is_x1, -1.0, 1.0).astype(np.float32)
    sin64 = (sinT[:, freq_idx] * sgn[None, :]).T
    cossin = np.stack(
        [np.tile(cos64, (2, 1)), np.tile(sin64, (2, 1))], axis=1
    ).astype(np.float16)  # [128, 2, T]

    qw2 = np.concatenate([q_norm_w[perm64], q_norm_w[perm64]])  # [128]
    kw2 = np.concatenate([k_norm_w[perm64], k_norm_w[perm64]])
    qkw = np.stack(
        [
            1.0 / (D * qw2 * qw2),
            1.0 / (D * kw2 * kw2),
            EPS / (qw2 * qw2),
            EPS / (kw2 * kw2),
        ],
        axis=1,
    ).astype(np.float32)  # [128, 4] = scale q/k, bias q/k

    tables = np.zeros((128, 256), dtype=np.float16)
    tables[0:64, 0:64] = 1.0
    tables[64:128, 64:128] = 1.0
    # causal mask for a diagonal 128x128 block: valid iff s <= t
    s_i = np.arange(128)[:, None]
    t_i = np.arange(128)[None, :]
    tables[:, 128:256] = (s_i <= t_i).astype(np.float16)

    woT = np.ascontiguousarray(wo.T).astype(np.float16)  # [hd, e]

    xT_b = [np.ascontiguousarray(x[b].T).astype(np.float16) for b in range(B)]

    # per-head row interleave for the q and k blocks of wqkv
    perm256 = np.concatenate([64 * hh + perm64 for hh in range(HL)])

    in_maps = []
    for c in range(N_CORES):
        b, g = c // 4, c % 4
        rq = slice(256 * g, 256 * g + 256)
        wsel = np.concatenate(
            [wqkv[rq][perm256], wqkv[C:][rq][perm256], wqkv[2 * C :][rq]], axis=0
        )  # [768, C]
        wqkvT = np.ascontiguousarray(wsel.T).astype(np.float16)
        in_maps.append(
            {
                "xT": xT_b[b],
                "wqkvT": wqkvT,
                "woT": woT,
                "cossin": cossin,
                "qkw": qkw,
                "tables": tables,
            }
        )
    return in_maps


def get_program():
    if "nc" not in _BUILD_CACHE:
        _BUILD_CACHE["nc"] = _build_program()
    return _BUILD_CACHE["nc"]


def kernel(x, wqkv, wo, q_norm_w, k_norm_w):
    nc = get_program()
    in_maps = _host_inputs(x, wqkv, wo, q_norm_w, k_norm_w)
    res = run_bass_kernel_spmd(nc, in_maps, core_ids=list(range(N_CORES)))
    full = np.empty((B, T, C), dtype=np.float32)
    for c in range(N_CORES):
        o = res.results[c]["out"]  # [B, TS, C] f16
        full[:, TS * c : TS * (c + 1), :] = np.asarray(o, dtype=np.float32)
    return full


# revision 58
# speedup vs baseline: 1.1335x; 1.0001x over previous
"""Distributed causal self-attention kernel for 8 Trainium2 NeuronCores.

Problem: B=2, T=2048, C=1024, H=16 heads, D=64 head dim.
    qkv = x @ wqkv.T; q,k = rmsnorm(q|k)*w; rope; causal attention; out @ wo.T

Sharding: core c handles batch b = c//4 and head group g = c%4 (4 heads).
Per core:
  - QKV projection for its (b, heads) in transposed [o, t] layout (q, k)
    plus natural [s, d] layout for v.  q and k are both packed 2 heads per
    128-partition tile; the per-head QK matmul contracts over K=64 via
    base-partition-64 operand slices (no zero padding).
  - RMSNorm across d via a block-diagonal ones-matmul that sums and
    broadcasts per head in one shot.  rsqrt is computed as exp(-0.5*ln(y))
    so every scalar-engine activation (Exp/Ln/Copy) lives in ONE hardware
    table set and no table reloads occur.
  - RoPE via a single cross-partition stream_shuffle: the head dim is laid
    out host-side so each rope partner pair sits 16 partitions apart inside
    one 32-partition quadrant; the sign-folded sin table makes
    out = x*cos + shuffle(x)*sin exact.
  - Causal attention per head in S^T = [s, t] layout.  ||q||=||k||=sqrt(D)
    after rmsnorm, so scores are bounded and softmax needs no max
    subtraction.  exp runs on psum score pairs; only true diagonal
    128x128 blocks are masked, by one triangular table (fully-masked
    score columns left of each s-chunk's first valid t-block are not
    even computed).
  - AV runs TRANSPOSED: for each 128-wide t-block, matmul(lhsT=es_block,
    rhs=v_chunk) accumulates [t, d+1] in psum with N=65 per matmul --
    about half the tensor-engine streaming cost of the [d, t] form.  The
    ones column of V gives the softmax denominator; the division is a
    per-partition reciprocal + broadcast multiply during psum evacuation.
  - ONE AllToAll (8 cores) swaps head-shards for T-shards in [t, h, d]
    layout; each core then DMA-TRANSPOSES (xbar dma transpose, ~14ns/tile)
    its received blocks into [hd, t] tiles and runs the wo projection with
    no further reduction.

Matmul operands are float16 (fp32 PSUM accumulation).
"""

import numpy as np

import concourse.bass as bass
import concourse.mybir as mybir
import concourse.tile as tile
from concourse import bacc
from concourse.bass_utils import run_bass_kernel_spmd

N_CORES = 8
B, T, C = 2, 2048, 1024
H, D = 16, 64          # global heads, head dim
HL = 4                 # heads per core
ROPE_THETA = 10000.0
EPS = 1e-6
KO = C // 128          # contraction chunks for C
NT = T // 512          # 512-wide t-chunks
NS = T // 128          # 128-wide s-chunks
TS = T // N_CORES      # t-rows owned per core after AllToAll (256)

F16 = mybir.dt.float16
F32 = mybir.dt.float32

_BUILD_CACHE = {}
SKIP_COLLECTIVE = False  # drop the AllToAll (for single-core TimelineSim)
N_COLLECTIVES = 1

# rope partner shuffle: swap 16-partition halves within each 32-partition
# quadrant (see _host_inputs for the matching weight-row interleave)
SWAP16 = [(i + 16) % 32 for i in range(32)]


def _build_program():
    nc = bacc.Bacc(
        "TRN2",
        target_bir_lowering=False,
        debug=False,
        enable_asserts=False,
        num_devices=N_CORES,
    )
    xT = nc.dram_tensor("xT", [C, T], F16, kind="ExternalInput").ap()
    wqkvT = nc.dram_tensor("wqkvT", [C, 3 * HL * D], F16, kind="ExternalInput").ap()
    woT = nc.dram_tensor("woT", [H * D, C], F16, kind="ExternalInput").ap()
    # rope tables: [:, 0] = cos, [:, 1] = sign-folded sin
    cossin = nc.dram_tensor("cossin", [128, 2, T], F16, kind="ExternalInput").ap()
    # ln scale/bias with the norm weight folded in ([:, 0:2] = scale q/k,
    # [:, 2:4] = bias q/k):
    # exp(-0.5*ln(sum*qkw_s + qkw_b)) == w * rsqrt(mean + eps) for w > 0
    qkw = nc.dram_tensor("qkw", [128, 4], F32, kind="ExternalInput").ap()
    # [:, 0:128] block-diag ones (rmsnorm sum), [:, 128:256] causal triangle
    tables = nc.dram_tensor("tables", [128, 256], F16, kind="ExternalInput").ap()
    out = nc.dram_tensor("out", [B, TS, C], F16, kind="ExternalOutput").ap()

    with tile.TileContext(nc) as tc:
        _emit(tc, xT, wqkvT, woT, cossin, qkw, tables, out)
    nc.compile()
    return nc


def _emit(tc, xT, wqkvT, woT, cossin, qkw, tables, out):
    nc = tc.nc
    Exp = mybir.ActivationFunctionType.Exp
    Ln = mybir.ActivationFunctionType.Ln

    import contextlib

    with contextlib.ExitStack() as ctx:
        const = ctx.enter_context(tc.tile_pool(name="const", bufs=1))
        work = ctx.enter_context(tc.tile_pool(name="work", bufs=4))
        work3 = ctx.enter_context(tc.tile_pool(name="work3", bufs=8))
        espool = ctx.enter_context(tc.tile_pool(name="espool", bufs=8))
        ps512 = ctx.enter_context(tc.tile_pool(name="ps512", bufs=2, space="PSUM"))
        pss2 = ctx.enter_context(tc.tile_pool(name="pss2", bufs=2, space="PSUM"))
        po4p = ctx.enter_context(tc.tile_pool(name="po4p", bufs=2, space="PSUM"))
        dram = ctx.enter_context(tc.tile_pool(name="dram", bufs=1, space="DRAM"))

        # ---- persistent SBUF tiles ----
        xT_sb = const.tile([128, KO, T], F16, tag="xT_sb")
        wqkvT_sb = const.tile([128, KO, 3 * HL * D], F16, tag="wqkvT_sb")
        woT_sb = const.tile([128, KO, C], F16, tag="woT_sb")
        cossin_sb = const.tile([128, 2, T], F16, tag="cossin_sb")
        qkw_sb = const.tile([128, 4], F32, tag="qkw_sb")
        tables_sb = const.tile([128, 256], F16, tag="tables_sb")
        cos2_sb = cossin_sb[:, 0, :]
        sin2_sb = cossin_sb[:, 1, :]
        ones_sb = tables_sb[:, 0:128]
        mtri_sb = tables_sb[:, 128:256]
        qT = [
            const.tile([128, T], F16, tag=f"qT{p}", name=f"qT{p}") for p in range(2)
        ]
        kT = [
            const.tile([128, T], F16, tag=f"kT{p}", name=f"kT{p}") for p in range(2)
        ]
        vt = const.tile([128, NS, HL * (D + 1)], F16, tag="vt")
        # one aT tile per (batch, t-slot) so each out-projection group only
        # depends on its own 4 transposes, not all 16
        aTq = [
            const.tile([128, KO, 128], F16, tag=f"aTq{i}", name=f"aTq{i}")
            for i in range(4)
        ]

        # collective buffer in [t, h, d] layout:
        # (dst core, t-slot of 128, 128 t, 4 heads, 64 d)
        cc = dram.tile([N_CORES, 2, 128, HL, D], F16, tag="cc")
        cc_o = dram.tile([N_CORES, 2, 128, HL, D], F16, tag="cc_o")

        # ---- input DMAs (batched to keep HWDGE serialization low; the
        # first 512 t-columns of x and the qkv weights come first, small
        # tables next, then the rest of x; woT last -- it is only needed
        # by the output projection) ----
        xT_r = xT.rearrange("(ko p) t -> p ko t", p=128)
        wq_r = wqkvT.rearrange("(ko p) o -> p ko o", p=128)
        # first 512 t-columns of x and the weights in 2-ko chunks so the
        # in-order PE can start the first projection accumulation while the
        # rest streams in
        for koq in range(4):
            ks = bass.ts(koq, 2)
            nc.sync.dma_start(xT_sb[:, ks, 0:512], xT_r[:, ks, 0:512])
            nc.sync.dma_start(wqkvT_sb[:, ks, :], wq_r[:, ks, :])
        nc.sync.dma_start(qkw_sb[:], qkw[:])
        nc.sync.dma_start(tables_sb[:], tables[:])
        nc.sync.dma_start(cossin_sb[:, :, 0:512], cossin[:, :, 0:512])
        for koh in range(2):
            ks = bass.ts(koh, 4)
            nc.sync.dma_start(xT_sb[:, ks, 512:1024], xT_r[:, ks, 512:1024])
        nc.sync.dma_start(cossin_sb[:, :, 512:T], cossin[:, :, 512:T])
        for koh in range(2):
            ks = bass.ts(koh, 4)
            nc.sync.dma_start(xT_sb[:, ks, 1024:T], xT_r[:, ks, 1024:T])
        nc.sync.dma_start(woT_sb[:], woT.rearrange("(ko p) e -> p ko e", p=128))

        # ones column in the V tiles (drives the softmax denominator)
        nc.vector.memset(
            vt[:].rearrange("p s (h c) -> p s h c", c=D + 1)[:, :, :, D : D + 1], 1.0
        )

        # Load activation-table set 6 (natural_log_exp_and_others) ONCE up
        # front: it contains every function this kernel uses (Exp, Ln, Copy,
        # Square), so the Bacc fixpoint pass never needs to insert another
        # (it would otherwise thrash between the ln-only and exp-only sets).
        nc.scalar.add_instruction(
            mybir.InstLoadActFuncSet(
                name=nc.get_next_instruction_name(),
                ins=[],
                outs=[],
                act_func_set_id=6,
            )
        )

        # ---- thunk machinery: the PE executes strictly in order, so small
        # PE work units (projection ko-chunks, the norm ones-matmul) are
        # queued and popped between attention pairs.  This fills the PE
        # while the scalar engine's exp gates each pair's AV matmuls, and
        # it naturally delays each norm matmul until its DVE-side sum of
        # squares is long done. ----
        from collections import deque

        # entries are (deadline_key, thunk); deadline_key = (ti, si) means
        # the thunk MUST have run before attention chunk `ti` emits the pair
        # containing s-chunk `si` (si = -1: before chunk ti's first pair).
        # Pops are allowed any time; flush_until forces overdue thunks.
        filler = deque()

        def pop_filler(n=1):
            for _ in range(n):
                if filler:
                    filler.popleft()[1]()

        def flush_until(key):
            while filler and filler[0][0] <= key:
                filler.popleft()[1]()

        def flush_filler():
            while filler:
                filler.popleft()[1]()

        def qk_thunks(ot, ti):
            """q/k projection + rmsnorm + rope for one head-pair o-tile and
            one 512-wide t-chunk, split into 3 PE-granular thunks."""
            is_q = ot < 2
            pair = ot % 2
            wcol = 0 if is_q else 1
            dest = qT[pair] if is_q else kT[pair]
            tsl = bass.ts(ti, 512)
            st = {}

            def mk_proj(k0):
                def tp():
                    if k0 == 0:
                        st["ps"] = ps512.tile([128, 512], F32, tag="ps512", name="ps")
                    ps = st["ps"]
                    for ko in range(k0, k0 + 2):
                        nc.tensor.matmul(
                            ps[:],
                            lhsT=wqkvT_sb[:, ko, bass.ts(ot, 128)],
                            rhs=xT_sb[:, ko, tsl],
                            start=(ko == 0),
                            stop=(ko == KO - 1),
                        )
                    if k0 + 2 == KO:
                        # Evacuate psum immediately (frees the bank for the
                        # next accumulation), run the chain from the f16 copy.
                        rawe = work3.tile([128, 512], F16, tag="rawe", name="rawe")
                        nc.vector.tensor_copy(rawe[:], ps[:])
                        # rope partner shuffle does not depend on the norm
                        # coefficient (cb is constant across partner rows, so
                        # shuffle(raw*cb) == shuffle(raw)*cb and cb folds
                        # into the cos/sin coefficient tiles in t3).
                        rsw = work3.tile([128, 512], F16, tag="rsw", name="rsw")
                        nc.vector.stream_shuffle(rsw[:], rawe[:], SWAP16)
                        sq = work3.tile([128, 512], F16, tag="sq", name="sq")
                        nc.vector.tensor_mul(sq[:], rawe[:], rawe[:])
                        st["rawe"], st["rsw"], st["sq"] = rawe, rsw, sq

                return tp

            def t3():
                # Per-head sum of squares, replicated across the head's 64
                # partitions by the block-diagonal ones lhsT in one matmul.
                # The norm weight is folded into the ln's per-partition
                # scale/bias; rsqrt(y) = exp(-0.5*ln(y)) keeps the scalar
                # engine inside ONE activation-table set (Exp/Ln/Copy).
                ms = ps512.tile([128, 512], F32, tag="ps512", name="ms")
                nc.tensor.matmul(
                    ms[:], lhsT=ones_sb[:], rhs=st["sq"][:], start=True, stop=True
                )
                lncb = work.tile([128, 512], F32, tag="lncb", name="lncb")
                nc.scalar.activation(
                    lncb[:],
                    ms[:],
                    Ln,
                    bias=qkw_sb[:, 2 + wcol : 3 + wcol],
                    scale=qkw_sb[:, wcol : wcol + 1],
                )
                cb = work.tile([128, 512], F16, tag="cb", name="cb")
                nc.scalar.activation(cb[:], lncb[:], Exp, scale=-0.5)
                # dest = rawe*(cb*cos) + shuffle(rawe)*(cb*sin)
                cbc = work.tile([128, 512], F16, tag="cbc", name="cbc")
                nc.vector.tensor_mul(cbc[:], cb[:], cos2_sb[:, tsl])
                cbs = work.tile([128, 512], F16, tag="cbs", name="cbs")
                nc.vector.tensor_mul(cbs[:], cb[:], sin2_sb[:, tsl])
                rot = work.tile([128, 512], F16, tag="rot", name="rot")
                nc.vector.tensor_mul(rot[:], st["rsw"][:], cbs[:])
                cosm = work.tile([128, 512], F16, tag="cosm", name="cosm")
                nc.vector.tensor_mul(cosm[:], st["rawe"][:], cbc[:])
                nc.vector.tensor_add(dest[:, tsl], cosm[:], rot[:])

            return [mk_proj(0), mk_proj(2), mk_proj(4), mk_proj(6), t3]

        def v_thunks(st_):
            vt_heads = vt[:].rearrange("p s (h c) -> p s h c", c=D + 1)
            state = {}

            def mk_v(k0):
                def tv():
                    if k0 == 0:
                        state["psv"] = ps512.tile(
                            [128, 512], F32, tag="ps512", name="psv"
                        )
                    psv = state["psv"]
                    for ko in range(k0, k0 + 4):
                        nc.tensor.matmul(
                            psv[:, 0 : HL * D],
                            lhsT=xT_sb[:, ko, bass.ts(st_, 128)],
                            rhs=wqkvT_sb[:, ko, 2 * HL * D : 3 * HL * D],
                            start=(ko == 0),
                            stop=(ko == KO - 1),
                        )
                    if k0 + 4 == KO:
                        nc.vector.tensor_copy(
                            vt_heads[:, st_, :, 0:D],
                            psv[:, 0 : HL * D].rearrange("p (h c) -> p h c", c=D),
                        )

                return tv

            return [mk_v(0), mk_v(4)]

        def emit_attn(h, ti):
            """attention for one head and one 512-wide t-chunk.  Scores come
            in s-chunk pairs sharing a [128, 2, 512] psum tile (one exp per
            pair); AV accumulates transposed [t, d+1] tiles per 128-wide
            t-block with N=65 matmuls.  The pair loop is software-pipelined:
            pair p's AV runs after pair p+1's QK (plus one filler thunk), so
            the exp latency hides behind real PE work."""
            pair = h // 2
            hrow = bass.ds(64 * (h % 2), 64)
            po4 = po4p.tile([128, 4, D + 1], F32, tag="po4", name="po4")
            n_si = 4 * ti + 4
            av_state = {"first": True}

            def po4_mm(es, u, tj, cut, si, stop):
                nc.tensor.matmul(
                    po4[:, tj, :],
                    lhsT=es[:, u, bass.ds(128 * tj - cut, 128)],
                    rhs=vt[:, si, bass.ds(h * (D + 1), D + 1)],
                    start=av_state["first"],
                    stop=stop,
                )
                av_state["first"] = False

            def emit_av(p):
                # transposed AV: accumulate [t, d+1] per 128-wide t-block.
                # The whole po4 bank is ONE psum accumulation group (psum
                # group start/stop is bank-granular); per-element
                # has_written bits make the first write to each t-block an
                # overwrite and later ones accumulates.  Blocks that need no
                # causal mask go first so only the diagonal blocks wait on
                # the gpsimd mask ops.
                (es, j, cut) = p
                si0 = 4 * ti + j
                for u in range(2):
                    si = si0 + u
                    for tj in range(max(0, j + u + 1), 4):
                        po4_mm(es, u, tj, cut, si, False)
                # mask ONLY the true diagonal 128x128 blocks, on gpsimd
                meng = nc.vector
                for u in range(2):
                    tj = j + u
                    if tj >= 0:
                        blk = es[:, u, bass.ds(128 * tj - cut, 128)]
                        meng.tensor_mul(blk, blk, mtri_sb[:])
                for u in range(2):
                    tj = j + u
                    if tj >= 0:
                        po4_mm(es, u, tj, cut, 4 * ti + j + u, tj == 3)

            pend = []
            for si0 in range(0, n_si, 2):
                flush_until((ti, h, si0 + 1))
                j = si0 - 4 * ti
                # the last (most-diagonal) pair only attends within the
                # upper half of the t-chunk -- the lower half is fully
                # masked zeros, so compute it at half width
                last = j == 2
                tw = 256 if last else 512
                cut = 512 - tw           # es col 0 == chunk-local t = cut
                toff = 512 * ti + cut
                ps2 = pss2.tile([128, 2, 512], F32, tag="pss2", name="ps2")
                for u in range(2):
                    # columns left of this s-chunk's first valid t-block are
                    # fully masked -- skip computing them.  (The stale psum
                    # they leave behind is exp'd but its es block is never
                    # read by any AV matmul.  ti=0 keeps full width so the
                    # first use of each psum buffer is fully initialized.)
                    lo = max(0, 128 * (si0 + u) - toff) if ti > 0 else 0
                    nc.tensor.matmul(
                        ps2[:, u, lo:tw],
                        lhsT=kT[pair][hrow, bass.ts(si0 + u, 128)],
                        rhs=qT[pair][hrow, bass.ds(toff + lo, tw - lo)],
                        start=True,
                        stop=True,
                    )
                es = espool.tile([128, 2, 512], F16, tag="es", name="es")
                nc.scalar.activation(
                    es[:, :, 0:tw], ps2[:, :, 0:tw], Exp, scale=1.0 / np.sqrt(D)
                )
                pend.append((es, j, cut))
                if len(pend) > 1:
                    pop_filler()
                    emit_av(pend.pop(0))
            while pend:
                pop_filler()
                emit_av(pend.pop(0))
            # softmax division on evacuation: per-partition reciprocal of
            # the ones-column, broadcast along d
            rec4 = work.tile([128, 4], F32, tag="rec4", name="rec4")
            nc.vector.reciprocal(
                rec4[:], po4[:, :, D : D + 1].rearrange("p a b -> p (a b)")
            )
            ob = work.tile([128, 4, D], F16, tag="ob", name="ob")
            nc.vector.tensor_mul(
                ob[:], po4[:, :, 0:D], rec4[:].broadcast_to([128, 4, D])
            )
            # one DMA: t-block tj goes to dst core (4*ti+tj)//2, slot
            # (4*ti+tj)%2 -- contiguous (dst, slot) blocks in cc.  Issued
            # via the gpsimd SWDGE queue: the SP queue's DMA counting
            # semaphore then never chains the tail transposes behind these
            # writes.
            ccr = cc[:].rearrange("dst slot t h d -> t (dst slot) h d")
            nc.sync.dma_start(ccr[:, bass.ds(4 * ti, 4), h, :], ob[:])

        def emit_a2a():
            if SKIP_COLLECTIVE:
                return
            nc.gpsimd.collective_compute(
                "AllToAll",
                mybir.AluOpType.bypass,
                replica_groups=[list(range(N_CORES))],
                ins=[cc.opt()],
                outs=[cc_o.opt()],
            )

        def emit_tail():
            # aT column b*KO+ko holds hd rows [128*ko, 128*ko+128) of batch
            # b = global heads {2*ko, 2*ko+1}: source core 4*b + g covers
            # columns b*KO + {2g, 2g+1}.  One xbar dma transpose per
            # (source core, t-slot) turns the received [128 t, 4 h, 64 d]
            # block into both aT column tiles at once; the out projection
            # for that (b, t-slot) follows immediately so PE work overlaps
            # the remaining transposes.  In SKIP_COLLECTIVE timing mode the
            # transposes read cc itself, which models the true
            # after-all-attention ordering without the collective.
            src_buf = cc if SKIP_COLLECTIVE else cc_o
            for b in range(B):
                for slot in range(2):
                    aT = aTq[2 * b + slot]
                    for g in range(4):
                        nc.sync.dma_start_transpose(
                            aT[:, bass.ds(2 * g, 2), :],
                            src_buf[4 * b + g, slot, :, :, :],
                        )
                    tt = slot
                    for ec in range(C // 512):
                        pout = ps512.tile([128, 512], F32, tag="ps512", name="pout")
                        for ko in range(KO):
                            nc.tensor.matmul(
                                pout[:],
                                lhsT=aT[:, ko, :],
                                rhs=woT_sb[:, ko, bass.ts(ec, 512)],
                                start=(ko == 0),
                                stop=(ko == KO - 1),
                            )
                        ob = work.tile([128, 512], F16, tag="obp", name="obp")
                        nc.vector.tensor_copy(ob[:], pout[:])
                        nc.sync.dma_start(
                            out[b, bass.ts(tt, 128), bass.ts(ec, 512)], ob[:]
                        )

        # ---- emission order: chunk-0 projections up front (norm matmuls
        # staggered one slot behind their projections), then per 512-wide
        # t-chunk the NEXT chunk's projection thunks are queued and popped
        # between attention pairs; then ONE AllToAll, the dma-transposed
        # aT loads, and the output projection ----
        ot_order = (0, 2, 1, 3)
        pre = [qk_thunks(ot, 0) for ot in ot_order]
        vpre = [v_thunks(st) for st in range(4)]
        for i in range(4):
            for tp in pre[i][:4]:
                tp()
            if i > 0:
                pre[i - 1][4]()
            vpre[i][0]()
            vpre[i][1]()
        pre[3][4]()
        for ti in range(NT):
            # queue the next chunk's projection thunks with deadlines;
            # attention pairs pop one between each score/AV step so the PE
            # never idles while the scalar engine's exp stream gates the
            # AVs.  Q tiles are due at the next chunk's first pair, but K
            # tiles and V chunks are only due at the pair that first reads
            # them -- so they spill INTO the next chunk's attention and keep
            # feeding the PE during the exp-bound final chunk.
            if ti + 1 < NT:
                t1 = ti + 1
                for h in range(HL):
                    filler.extend(((t1, 0, -1), t) for t in qk_thunks(ot_order[h], t1))
                    filler.extend(((t1, 0, -1), t) for t in v_thunks(4 * t1 + h))
            for h in range(HL):
                emit_attn(h, ti)
            flush_filler()
        emit_a2a()
        emit_tail()


def _host_inputs(x, wqkv, wo, q_norm_w, k_norm_w):
    """Build the per-core input maps (all host-side prep is layout/dtype only)."""
    x = np.asarray(x, dtype=np.float32)
    wqkv = np.asarray(wqkv, dtype=np.float32)
    wo = np.asarray(wo, dtype=np.float32)
    q_norm_w = np.asarray(q_norm_w, dtype=np.float32)
    k_norm_w = np.asarray(k_norm_w, dtype=np.float32)

    # head-dim interleave: rope partner pairs (x1_i, x2_i) sit 16 partitions
    # apart within one 32-partition quadrant, so ONE stream_shuffle (swap
    # 16-halves per quadrant) aligns every partner.  perm64[new] = old.
    perm64 = np.concatenate(
        [np.arange(0, 16), np.arange(32, 48), np.arange(16, 32), np.arange(48, 64)]
    )
    # rope frequency index and partner-sign per (new) position
    j = np.arange(64)
    q32, r32 = j // 32, j % 32
    freq_idx = 16 * q32 + (r32 % 16)          # pair index i in [0, 32)
    is_x1 = r32 < 16                          # rows holding x1_i

    inv_freq = (
        1.0 / (ROPE_THETA ** (np.arange(0, D, 2, dtype=np.float32) / D))
    ).astype(np.float32)
    freqs = np.arange(T, dtype=np.float32)[:, None] * inv_freq[None, :]  # [T, 32]
    cosT = np.cos(freqs).astype(np.float32)  # [T, 32]
    sinT = np.sin(freqs).astype(np.float32)
    # per-partition tables for a 2-head (128-row) o-tile
    cos64 = cosT[:, freq_idx].T              # [64, T]
    sgn = np.where(is_x1, -1.0, 1.0).astype(np.float32)
    sin64 = (sinT[:, freq_idx] * sgn[None, :]).T
    cossin = np.stack(
        [np.tile(cos64, (2, 1)), np.tile(sin64, (2, 1))], axis=1
    ).astype(np.float16)  # [128, 2, T]

    qw2 = np.concatenate([q_norm_w[perm64], q_norm_w[perm64]])  # [128]
    kw2 = np.concatenate([k_norm_w[perm64], k_norm_w[perm64]])
    qkw = np.stack(
        [
            1.0 / (D * qw2 * qw2),
            1.0 / (D * kw2 * kw2),
            EPS / (qw2 * qw2),
            EPS / (kw2 * kw2),
        ],
        axis=1,
    ).astype(np.float32)  # [128, 4] = scale q/k, bias q/k

    tables = np.zeros((128, 256), dtype=np.float16)
    tables[0:64, 0:64] = 1.0
    tables[64:128, 64:128] = 1.0
    # causal mask for a diagonal 128x128 block: valid iff s <= t
    s_i = np.arange(128)[:, None]
    t_i = np.arange(128)[None, :]
    tables[:, 128:256] = (s_i <= t_i).astype(np.float16)

    woT = np.ascontiguousarray(wo.T).astype(np.float16)  # [hd, e]

    xT_b = [np.ascontiguousarray(x[b].T).astype(np.float16) for b in range(B)]

    # per-head row interleave for the q and k blocks of wqkv
    perm256 = np.concatenate([64 * hh + perm64 for hh in range(HL)])

    in_maps = []
    for c in range(N_CORES):
        b, g = c // 4, c % 4
        rq = slice(256 * g, 256 * g + 256)
        wsel = np.concatenate(
            [wqkv[rq][perm256], wqkv[C:][rq][perm256], wqkv[2 * C :][rq]], axis=0
        )  # [768, C]
        wqkvT = np.ascontiguousarray(wsel.T).astype(np.float16)
        in_maps.append(
            {
                "xT": xT_b[b],
                "wqkvT": wqkvT,
                "woT": woT,
                "cossin": cossin,
                "qkw": qkw,
                "tables": tables,
            }
        )
    return in_maps


def get_program():
    if "nc" not in _BUILD_CACHE:
        _BUILD_CACHE["nc"] = _build_program()
    return _BUILD_CACHE["nc"]


def kernel(x, wqkv, wo, q_norm_w, k_norm_w):
    nc = get_program()
    in_maps = _host_inputs(x, wqkv, wo, q_norm_w, k_norm_w)
    res = run_bass_kernel_spmd(nc, in_maps, core_ids=list(range(N_CORES)))
    full = np.empty((B, T, C), dtype=np.float32)
    for c in range(N_CORES):
        o = res.results[c]["out"]  # [B, TS, C] f16
        full[:, TS * c : TS * (c + 1), :] = np.asarray(o, dtype=np.float32)
    return full


# revision 81
# speedup vs baseline: 1.1556x; 1.0195x over previous
"""Distributed causal self-attention kernel for 8 Trainium2 NeuronCores.

Problem: B=2, T=2048, C=1024, H=16 heads, D=64 head dim.
    qkv = x @ wqkv.T; q,k = rmsnorm(q|k)*w; rope; causal attention; out @ wo.T

Sharding: core c handles batch b = c//4 and head group g = c%4 (4 heads).
Per core:
  - QKV projection for its (b, heads) in transposed [o, t] layout (q, k)
    plus natural [s, d] layout for v.  q and k are both packed 2 heads per
    128-partition tile; the per-head QK matmul contracts over K=64 via
    base-partition-64 operand slices (no zero padding).
  - RMSNorm across d via a block-diagonal ones-matmul that sums and
    broadcasts per head in one shot.  rsqrt is computed as exp(-0.5*ln(y))
    so every scalar-engine activation (Exp/Ln/Copy) lives in ONE hardware
    table set and no table reloads occur.
  - RoPE via a single cross-partition stream_shuffle: the head dim is laid
    out host-side so each rope partner pair sits 16 partitions apart inside
    one 32-partition quadrant; the sign-folded sin table makes
    out = x*cos + shuffle(x)*sin exact.
  - Causal attention per head in S^T = [s, t] layout.  ||q||=||k||=sqrt(D)
    after rmsnorm, so scores are bounded and softmax needs no max
    subtraction.  exp runs on psum score pairs; only true diagonal
    128x128 blocks are masked, by one triangular table (fully-masked
    score columns left of each s-chunk's first valid t-block are not
    even computed).
  - AV runs TRANSPOSED: for each 128-wide t-block, matmul(lhsT=es_block,
    rhs=v_chunk) accumulates [t, d+1] in psum with N=65 per matmul --
    about half the tensor-engine streaming cost of the [d, t] form.  The
    ones column of V gives the softmax denominator; the division is a
    per-partition reciprocal + broadcast multiply during psum evacuation.
  - ONE AllToAll (8 cores) swaps head-shards for T-shards in [t, h, d]
    layout; each core then DMA-TRANSPOSES (xbar dma transpose, ~14ns/tile)
    its received blocks into [hd, t] tiles and runs the wo projection with
    no further reduction.

Matmul operands are float16 (fp32 PSUM accumulation).
"""

import numpy as np

import concourse.bass as bass
import concourse.mybir as mybir
import concourse.tile as tile
from concourse import bacc
from concourse.bass_utils import run_bass_kernel_spmd

N_CORES = 8
B, T, C = 2, 2048, 1024
H, D = 16, 64          # global heads, head dim
HL = 4                 # heads per core
ROPE_THETA = 10000.0
EPS = 1e-6
KO = C // 128          # contraction chunks for C
NT = T // 512          # 512-wide t-chunks
NS = T // 128          # 128-wide s-chunks
TS = T // N_CORES      # t-rows owned per core after AllToAll (256)

F16 = mybir.dt.float16
F32 = mybir.dt.float32

_BUILD_CACHE = {}
SKIP_COLLECTIVE = False  # drop the AllToAll (for single-core TimelineSim)
N_COLLECTIVES = 1

# rope partner shuffle: swap 16-partition halves within each 32-partition
# quadrant (see _host_inputs for the matching weight-row interleave)
SWAP16 = [(i + 16) % 32 for i in range(32)]


def _build_program():
    nc = bacc.Bacc(
        "TRN2",
        target_bir_lowering=False,
        debug=False,
        enable_asserts=False,
        num_devices=N_CORES,
    )
    xT = nc.dram_tensor("xT", [C, T], F16, kind="ExternalInput").ap()
    wqkvT = nc.dram_tensor("wqkvT", [C, 3 * HL * D], F16, kind="ExternalInput").ap()
    woT = nc.dram_tensor("woT", [H * D, C], F16, kind="ExternalInput").ap()
    # rope tables: [:, 0] = cos, [:, 1] = sign-folded sin
    cossin = nc.dram_tensor("cossin", [128, 2, T], F16, kind="ExternalInput").ap()
    # ln scale/bias with the norm weight folded in ([:, 0:2] = scale q/k,
    # [:, 2:4] = bias q/k):
    # exp(-0.5*ln(sum*qkw_s + qkw_b)) == w * rsqrt(mean + eps) for w > 0
    qkw = nc.dram_tensor("qkw", [128, 4], F32, kind="ExternalInput").ap()
    # [:, 0:128] block-diag ones (rmsnorm sum), [:, 128:256] causal triangle
    tables = nc.dram_tensor("tables", [128, 256], F16, kind="ExternalInput").ap()
    out = nc.dram_tensor("out", [B, TS, C], F16, kind="ExternalOutput").ap()

    with tile.TileContext(nc) as tc:
        _emit(tc, xT, wqkvT, woT, cossin, qkw, tables, out)
    nc.compile()
    return nc


def _emit(tc, xT, wqkvT, woT, cossin, qkw, tables, out):
    nc = tc.nc
    Exp = mybir.ActivationFunctionType.Exp
    Ln = mybir.ActivationFunctionType.Ln

    import contextlib

    with contextlib.ExitStack() as ctx:
        const = ctx.enter_context(tc.tile_pool(name="const", bufs=1))
        work = ctx.enter_context(tc.tile_pool(name="work", bufs=6))
        work3 = ctx.enter_context(tc.tile_pool(name="work3", bufs=8))
        espool = ctx.enter_context(tc.tile_pool(name="espool", bufs=20))
        ps512 = ctx.enter_context(tc.tile_pool(name="ps512", bufs=2, space="PSUM"))
        pss2 = ctx.enter_context(tc.tile_pool(name="pss2", bufs=2, space="PSUM"))
        po4p = ctx.enter_context(tc.tile_pool(name="po4p", bufs=2, space="PSUM"))
        dram = ctx.enter_context(tc.tile_pool(name="dram", bufs=1, space="DRAM"))

        # ---- persistent SBUF tiles ----
        xT_sb = const.tile([128, KO, T], F16, tag="xT_sb")
        wqkvT_sb = const.tile([128, KO, 3 * HL * D], F16, tag="wqkvT_sb")
        woT_sb = const.tile([128, KO, C], F16, tag="woT_sb")
        cossin_sb = const.tile([128, 2, T], F16, tag="cossin_sb")
        qkw_sb = const.tile([128, 4], F32, tag="qkw_sb")
        tables_sb = const.tile([128, 256], F16, tag="tables_sb")
        cos2_sb = cossin_sb[:, 0, :]
        sin2_sb = cossin_sb[:, 1, :]
        ones_sb = tables_sb[:, 0:128]
        mtri_sb = tables_sb[:, 128:256]
        qT = [
            const.tile([128, T], F16, tag=f"qT{p}", name=f"qT{p}") for p in range(2)
        ]
        kT = [
            const.tile([128, T], F16, tag=f"kT{p}", name=f"kT{p}") for p in range(2)
        ]
        vt = const.tile([128, NS, HL * (D + 1)], F16, tag="vt")
        # one aT tile per (batch, t-slot) so each out-projection group only
        # depends on its own 4 transposes, not all 16
        aTq = [
            const.tile([128, KO, 128], F16, tag=f"aTq{i}", name=f"aTq{i}")
            for i in range(4)
        ]

        # collective buffer in [t, h, d] layout:
        # (dst core, t-slot of 128, 128 t, 4 heads, 64 d)
        cc = dram.tile([N_CORES, 2, 128, HL, D], F16, tag="cc")
        cc_o = dram.tile([N_CORES, 2, 128, HL, D], F16, tag="cc_o")

        # ---- input DMAs (batched to keep HWDGE serialization low; the
        # first 512 t-columns of x and the qkv weights come first, small
        # tables next, then the rest of x; woT last -- it is only needed
        # by the output projection) ----
        xT_r = xT.rearrange("(ko p) t -> p ko t", p=128)
        wq_r = wqkvT.rearrange("(ko p) o -> p ko o", p=128)
        # first 512 t-columns of x and the weights in 2-ko chunks so the
        # in-order PE can start the first projection accumulation while the
        # rest streams in
        for koq in range(4):
            ks = bass.ts(koq, 2)
            nc.sync.dma_start(xT_sb[:, ks, 0:512], xT_r[:, ks, 0:512])
            nc.sync.dma_start(wqkvT_sb[:, ks, :], wq_r[:, ks, :])
        nc.sync.dma_start(qkw_sb[:], qkw[:])
        nc.sync.dma_start(tables_sb[:], tables[:])
        nc.sync.dma_start(cossin_sb[:, :, 0:512], cossin[:, :, 0:512])
        for koh in range(2):
            ks = bass.ts(koh, 4)
            nc.sync.dma_start(xT_sb[:, ks, 512:1024], xT_r[:, ks, 512:1024])
        nc.sync.dma_start(cossin_sb[:, :, 512:T], cossin[:, :, 512:T])
        for koh in range(2):
            ks = bass.ts(koh, 4)
            nc.sync.dma_start(xT_sb[:, ks, 1024:T], xT_r[:, ks, 1024:T])
        nc.sync.dma_start(woT_sb[:], woT.rearrange("(ko p) e -> p ko e", p=128))

        # ones column in the V tiles (drives the softmax denominator)
        nc.vector.memset(
            vt[:].rearrange("p s (h c) -> p s h c", c=D + 1)[:, :, :, D : D + 1], 1.0
        )

        # Load activation-table set 6 (natural_log_exp_and_others) ONCE up
        # front: it contains every function this kernel uses (Exp, Ln, Copy,
        # Square), so the Bacc fixpoint pass never needs to insert another
        # (it would otherwise thrash between the ln-only and exp-only sets).
        nc.scalar.add_instruction(
            mybir.InstLoadActFuncSet(
                name=nc.get_next_instruction_name(),
                ins=[],
                outs=[],
                act_func_set_id=6,
            )
        )

        # ---- thunk machinery: the PE executes strictly in order, so small
        # PE work units (projection ko-chunks, the norm ones-matmul) are
        # queued and popped between attention pairs.  This fills the PE
        # while the scalar engine's exp gates each pair's AV matmuls, and
        # it naturally delays each norm matmul until its DVE-side sum of
        # squares is long done. ----
        from collections import deque

        # entries are (deadline_key, thunk); deadline_key = (ti, si) means
        # the thunk MUST have run before attention chunk `ti` emits the pair
        # containing s-chunk `si` (si = -1: before chunk ti's first pair).
        # Pops are allowed any time; flush_until forces overdue thunks.
        filler = deque()

        def pop_filler(n=1):
            for _ in range(n):
                if filler:
                    filler.popleft()[1]()

        def flush_until(key):
            while filler and filler[0][0] <= key:
                filler.popleft()[1]()

        def flush_filler():
            while filler:
                filler.popleft()[1]()

        def qk_thunks(ot, ti):
            """q/k projection + rmsnorm + rope for one head-pair o-tile and
            one 512-wide t-chunk, split into 3 PE-granular thunks."""
            is_q = ot < 2
            pair = ot % 2
            wcol = 0 if is_q else 1
            dest = qT[pair] if is_q else kT[pair]
            tsl = bass.ts(ti, 512)
            st = {}

            def mk_proj(k0):
                def tp():
                    if k0 == 0:
                        st["ps"] = ps512.tile([128, 512], F32, tag="ps512", name="ps")
                    ps = st["ps"]
                    for ko in range(k0, k0 + 2):
                        nc.tensor.matmul(
                            ps[:],
                            lhsT=wqkvT_sb[:, ko, bass.ts(ot, 128)],
                            rhs=xT_sb[:, ko, tsl],
                            start=(ko == 0),
                            stop=(ko == KO - 1),
                        )
                    if k0 + 2 == KO:
                        # Evacuate psum immediately (frees the bank for the
                        # next accumulation), run the chain from the f16 copy.
                        rawe = work3.tile([128, 512], F16, tag="rawe", name="rawe")
                        nc.vector.tensor_copy(rawe[:], ps[:])
                        # rope partner shuffle does not depend on the norm
                        # coefficient (cb is constant across partner rows, so
                        # shuffle(raw*cb) == shuffle(raw)*cb and cb folds
                        # into the cos/sin coefficient tiles in t3).

                        sq = work3.tile([128, 512], F16, tag="sq", name="sq")
                        nc.vector.tensor_mul(sq[:], rawe[:], rawe[:])
                        st["rawe"], st["sq"] = rawe, sq

                return tp

            def t3():
                # Per-head sum of squares, replicated across the head's 64
                # partitions by the block-diagonal ones lhsT in one matmul.
                # The norm weight is folded into the ln's per-partition
                # scale/bias; rsqrt(y) = exp(-0.5*ln(y)) keeps the scalar
                # engine inside ONE activation-table set (Exp/Ln/Copy).
                ms = ps512.tile([128, 512], F32, tag="ps512", name="ms")
                nc.tensor.matmul(
                    ms[:], lhsT=ones_sb[:], rhs=st["sq"][:], start=True, stop=True
                )
                lncb = work.tile([128, 512], F32, tag="lncb", name="lncb")
                nc.scalar.activation(
                    lncb[:],
                    ms[:],
                    Ln,
                    bias=qkw_sb[:, 2 + wcol : 3 + wcol],
                    scale=qkw_sb[:, wcol : wcol + 1],
                )
                cb = work.tile([128, 512], F16, tag="cb", name="cb")
                nc.scalar.activation(cb[:], lncb[:], Exp, scale=-0.5)
                # dest = rawe*(cb*cos) + shuffle(rawe)*(cb*sin)
                rawn = work.tile([128, 512], F16, tag="rawn", name="rawn")
                nc.vector.tensor_mul(rawn[:], st["rawe"][:], cb[:])
                rsw = work.tile([128, 512], F16, tag="rsw", name="rsw")
                nc.vector.stream_shuffle(rsw[:], rawn[:], SWAP16)
                rot = work.tile([128, 512], F16, tag="rot", name="rot")
                nc.vector.tensor_mul(rot[:], rsw[:], sin2_sb[:, tsl])
                cosm = work.tile([128, 512], F16, tag="cosm", name="cosm")
                nc.vector.tensor_mul(cosm[:], rawn[:], cos2_sb[:, tsl])
                nc.vector.tensor_add(dest[:, tsl], cosm[:], rot[:])

            return [mk_proj(0), mk_proj(2), mk_proj(4), mk_proj(6), t3]

        def v_thunks(st_):
            vt_heads = vt[:].rearrange("p s (h c) -> p s h c", c=D + 1)
            state = {}

            def mk_v(k0):
                def tv():
                    if k0 == 0:
                        state["psv"] = ps512.tile(
                            [128, 512], F32, tag="ps512", name="psv"
                        )
                    psv = state["psv"]
                    for ko in range(k0, k0 + 4):
                        nc.tensor.matmul(
                            psv[:, 0 : HL * D],
                            lhsT=xT_sb[:, ko, bass.ts(st_, 128)],
                            rhs=wqkvT_sb[:, ko, 2 * HL * D : 3 * HL * D],
                            start=(ko == 0),
                            stop=(ko == KO - 1),
                        )
                    if k0 + 4 == KO:
                        nc.vector.tensor_copy(
                            vt_heads[:, st_, :, 0:D],
                            psv[:, 0 : HL * D].rearrange("p (h c) -> p h c", c=D),
                        )

                return tv

            return [mk_v(0), mk_v(4)]

        def emit_attn(h, ti):
            """attention for one head and one 512-wide t-chunk.  Scores come
            in s-chunk pairs sharing a [128, 2, 512] psum tile (one exp per
            pair); AV accumulates transposed [t, d+1] tiles per 128-wide
            t-block with N=65 matmuls.  The pair loop is software-pipelined:
            pair p's AV runs after pair p+1's QK (plus one filler thunk), so
            the exp latency hides behind real PE work."""
            pair = h // 2
            hrow = bass.ds(64 * (h % 2), 64)
            po4 = po4p.tile([128, 4, D + 1], F32, tag="po4", name="po4")
            n_si = 4 * ti + 4
            av_state = {"first": True}

            def po4_mm(es, u, tj, cut, si, stop):
                nc.tensor.matmul(
                    po4[:, tj, :],
                    lhsT=es[:, u, bass.ds(128 * tj - cut, 128)],
                    rhs=vt[:, si, bass.ds(h * (D + 1), D + 1)],
                    start=av_state["first"],
                    stop=stop,
                )
                av_state["first"] = False

            def emit_av(p):
                # transposed AV: accumulate [t, d+1] per 128-wide t-block.
                # The whole po4 bank is ONE psum accumulation group (psum
                # group start/stop is bank-granular); per-element
                # has_written bits make the first write to each t-block an
                # overwrite and later ones accumulates.  Blocks that need no
                # causal mask go first so only the diagonal blocks wait on
                # the gpsimd mask ops.
                (es, j, cut) = p
                si0 = 4 * ti + j
                for u in range(2):
                    si = si0 + u
                    for tj in range(max(0, j + u + 1), 4):
                        po4_mm(es, u, tj, cut, si, False)
                # mask ONLY the true diagonal 128x128 blocks, on gpsimd
                meng = nc.vector
                for u in range(2):
                    tj = j + u
                    if tj >= 0:
                        blk = es[:, u, bass.ds(128 * tj - cut, 128)]
                        meng.tensor_mul(blk, blk, mtri_sb[:])
                for u in range(2):
                    tj = j + u
                    if tj >= 0:
                        po4_mm(es, u, tj, cut, 4 * ti + j + u, tj == 3)

            pend = []
            for si0 in range(0, n_si, 2):
                flush_until((ti, h, si0 + 1))
                j = si0 - 4 * ti
                # the last (most-diagonal) pair only attends within the
                # upper half of the t-chunk -- the lower half is fully
                # masked zeros, so compute it at half width
                last = j == 2
                tw = 256 if last else 512
                cut = 512 - tw           # es col 0 == chunk-local t = cut
                toff = 512 * ti + cut
                ps2 = pss2.tile([128, 2, 512], F32, tag="pss2", name="ps2")
                for u in range(2):
                    nc.tensor.matmul(
                        ps2[:, u, 0:tw],
                        lhsT=kT[pair][hrow, bass.ts(si0 + u, 128)],
                        rhs=qT[pair][hrow, bass.ds(toff, tw)],
                        start=True,
                        stop=True,
                    )
                es = espool.tile([128, 2, 512], F16, tag="es", name="es")
                nc.scalar.activation(
                    es[:, :, 0:tw], ps2[:, :, 0:tw], Exp, scale=1.0 / np.sqrt(D)
                )
                pend.append((es, j, cut))
                if len(pend) > 1:
                    pop_filler()
                    emit_av(pend.pop(0))
            while pend:
                pop_filler()
                emit_av(pend.pop(0))
            # softmax division on evacuation: per-partition reciprocal of
            # the ones-column, broadcast along d
            rec4 = work.tile([128, 4], F32, tag="rec4", name="rec4")
            nc.vector.reciprocal(
                rec4[:], po4[:, :, D : D + 1].rearrange("p a b -> p (a b)")
            )
            ob = work.tile([128, 4, D], F16, tag="ob", name="ob")
            nc.vector.tensor_mul(
                ob[:], po4[:, :, 0:D], rec4[:].broadcast_to([128, 4, D])
            )
            # one DMA: t-block tj goes to dst core (4*ti+tj)//2, slot
            # (4*ti+tj)%2 -- contiguous (dst, slot) blocks in cc.  Issued
            # via the gpsimd SWDGE queue: the SP queue's DMA counting
            # semaphore then never chains the tail transposes behind these
            # writes.
            ccr = cc[:].rearrange("dst slot t h d -> t (dst slot) h d")
            nc.sync.dma_start(ccr[:, bass.ds(4 * ti, 4), h, :], ob[:])

        def emit_a2a():
            if SKIP_COLLECTIVE:
                return
            nc.gpsimd.collective_compute(
                "AllToAll",
                mybir.AluOpType.bypass,
                replica_groups=[list(range(N_CORES))],
                ins=[cc.opt()],
                outs=[cc_o.opt()],
            )

        def emit_tail():
            # aT column b*KO+ko holds hd rows [128*ko, 128*ko+128) of batch
            # b = global heads {2*ko, 2*ko+1}: source core 4*b + g covers
            # columns b*KO + {2g, 2g+1}.  One xbar dma transpose per
            # (source core, t-slot) turns the received [128 t, 4 h, 64 d]
            # block into both aT column tiles at once; the out projection
            # for that (b, t-slot) follows immediately so PE work overlaps
            # the remaining transposes.  In SKIP_COLLECTIVE timing mode the
            # transposes read cc itself, which models the true
            # after-all-attention ordering without the collective.
            src_buf = cc if SKIP_COLLECTIVE else cc_o
            for b in range(B):
                for slot in range(2):
                    aT = aTq[2 * b + slot]
                    for g in range(4):
                        nc.sync.dma_start_transpose(
                            aT[:, bass.ds(2 * g, 2), :],
                            src_buf[4 * b + g, slot, :, :, :],
                        )
                    tt = slot
                    for ec in range(C // 512):
                        pout = ps512.tile([128, 512], F32, tag="ps512", name="pout")
                        for ko in range(KO):
                            nc.tensor.matmul(
                                pout[:],
                                lhsT=aT[:, ko, :],
                                rhs=woT_sb[:, ko, bass.ts(ec, 512)],
                                start=(ko == 0),
                                stop=(ko == KO - 1),
                            )
                        ob = work.tile([128, 512], F16, tag="obp", name="obp")
                        nc.vector.tensor_copy(ob[:], pout[:])
                        nc.sync.dma_start(
                            out[b, bass.ts(tt, 128), bass.ts(ec, 512)], ob[:]
                        )

        # ---- emission order: chunk-0 projections up front (norm matmuls
        # staggered one slot behind their projections), then per 512-wide
        # t-chunk the NEXT chunk's projection thunks are queued and popped
        # between attention pairs; then ONE AllToAll, the dma-transposed
        # aT loads, and the output projection ----
        ot_order = (2, 3, 0, 1)
        pre = [qk_thunks(ot, 0) for ot in ot_order]
        vpre = [v_thunks(st) for st in range(4)]
        for i in range(4):
            for tp in pre[i][:4]:
                tp()
            if i > 0:
                pre[i - 1][4]()
            vpre[i][0]()
            vpre[i][1]()
        pre[3][4]()
        for ti in range(NT):
            # queue the next chunk's projection thunks with deadlines;
            # attention pairs pop one between each score/AV step so the PE
            # never idles while the scalar engine's exp stream gates the
            # AVs.  Q tiles are due at the next chunk's first pair, but K
            # tiles and V chunks are only due at the pair that first reads
            # them -- so they spill INTO the next chunk's attention and keep
            # feeding the PE during the exp-bound final chunk.
            if ti + 1 < NT:
                t1 = ti + 1
                for h in range(HL):
                    filler.extend(((t1, 0, -1), t) for t in qk_thunks(ot_order[h], t1))
                    filler.extend(((t1, 0, -1), t) for t in v_thunks(4 * t1 + h))
            for h in range(HL):
                emit_attn(h, ti)
            flush_filler()
        emit_a2a()
        emit_tail()


def _host_inputs(x, wqkv, wo, q_norm_w, k_norm_w):
    """Build the per-core input maps (all host-side prep is layout/dtype only)."""
    x = np.asarray(x, dtype=np.float32)
    wqkv = np.asarray(wqkv, dtype=np.float32)
    wo = np.asarray(wo, dtype=np.float32)
    q_norm_w = np.asarray(q_norm_w, dtype=np.float32)
    k_norm_w = np.asarray(k_norm_w, dtype=np.float32)

    # head-dim interleave: rope partner pairs (x1_i, x2_i) sit 16 partitions
    # apart within one 32-partition quadrant, so ONE stream_shuffle (swap
    # 16-halves per quadrant) aligns every partner.  perm64[new] = old.
    perm64 = np.concatenate(
        [np.arange(0, 16), np.arange(32, 48), np.arange(16, 32), np.arange(48, 64)]
    )
    # rope frequency index and partner-sign per (new) position
    j = np.arange(64)
    q32, r32 = j // 32, j % 32
    freq_idx = 16 * q32 + (r32 % 16)          # pair index i in [0, 32)
    is_x1 = r32 < 16                          # rows holding x1_i

    inv_freq = (
        1.0 / (ROPE_THETA ** (np.arange(0, D, 2, dtype=np.float32) / D))
    ).astype(np.float32)
    freqs = np.arange(T, dtype=np.float32)[:, None] * inv_freq[None, :]  # [T, 32]
    cosT = np.cos(freqs).astype(np.float32)  # [T, 32]
    sinT = np.sin(freqs).astype(np.float32)
    # per-partition tables for a 2-head (128-row) o-tile
    cos64 = cosT[:, freq_idx].T              # [64, T]
    sgn = np.where(is_x1, -1.0, 1.0).astype(np.float32)
    sin64 = (sinT[:, freq_idx] * sgn[None, :]).T
    cossin = np.stack(
        [np.tile(cos64, (2, 1)), np.tile(sin64, (2, 1))], axis=1
    ).astype(np.float16)  # [128, 2, T]

    qw2 = np.concatenate([q_norm_w[perm64], q_norm_w[perm64]])  # [128]
    kw2 = np.concatenate([k_norm_w[perm64], k_norm_w[perm64]])
    qkw = np.stack(
        [
            1.0 / (D * qw2 * qw2),
            1.0 / (D * kw2 * kw2),
            EPS / (qw2 * qw2),
            EPS / (kw2 * kw2),
        ],
        axis=1,
    ).astype(np.float32)  # [128, 4] = scale q/k, bias q/k

    tables = np.zeros((128, 256), dtype=np.float16)
    tables[0:64, 0:64] = 1.0
    tables[64:128, 64:128] = 1.0
    # causal mask for a diagonal 128x128 block: valid iff s <= t
    s_i = np.arange(128)[:, None]
    t_i = np.arange(128)[None, :]
    tables[:, 128:256] = (s_i <= t_i).astype(np.float16)

    woT = np.ascontiguousarray(wo.T).astype(np.float16)  # [hd, e]

    xT_b = [np.ascontiguousarray(x[b].T).astype(np.float16) for b in range(B)]

    # per-head row interleave for the q and k blocks of wqkv
    perm256 = np.concatenate([64 * hh + perm64 for hh in range(HL)])

    in_maps = []
    for c in range(N_CORES):
        b, g = c // 4, c % 4
        rq = slice(256 * g, 256 * g + 256)
        wsel = np.concatenate(
            [wqkv[rq][perm256], wqkv[C:][rq][perm256], wqkv[2 * C :][rq]], axis=0
        )  # [768, C]
        wqkvT = np.ascontiguousarray(wsel.T).astype(np.float16)
        in_maps.append(
            {
                "xT": xT_b[b],
                "wqkvT": wqkvT,
                "woT": woT,
                "cossin": cossin,
                "qkw": qkw,
                "tables": tables,
            }
        )
    return in_maps


def get_program():
    if "nc" not in _BUILD_CACHE:
        _BUILD_CACHE["nc"] = _build_program()
    return _BUILD_CACHE["nc"]


def kernel(x, wqkv, wo, q_norm_w, k_norm_w):
    nc = get_program()
    in_maps = _host_inputs(x, wqkv, wo, q_norm_w, k_norm_w)
    res = run_bass_kernel_spmd(nc, in_maps, core_ids=list(range(N_CORES)))
    full = np.empty((B, T, C), dtype=np.float32)
    for c in range(N_CORES):
        o = res.results[c]["out"]  # [B, TS, C] f16
        full[:, TS * c : TS * (c + 1), :] = np.asarray(o, dtype=np.float32)
    return full


# revision 84
# speedup vs baseline: 1.1744x; 1.0163x over previous
"""Distributed causal self-attention kernel for 8 Trainium2 NeuronCores.

Problem: B=2, T=2048, C=1024, H=16 heads, D=64 head dim.
    qkv = x @ wqkv.T; q,k = rmsnorm(q|k)*w; rope; causal attention; out @ wo.T

Sharding: core c handles batch b = c//4 and head group g = c%4 (4 heads).
Per core:
  - QKV projection for its (b, heads) in transposed [o, t] layout (q, k)
    plus natural [s, d] layout for v.  q and k are both packed 2 heads per
    128-partition tile; the per-head QK matmul contracts over K=64 via
    base-partition-64 operand slices (no zero padding).
  - RMSNorm across d via a block-diagonal ones-matmul that sums and
    broadcasts per head in one shot.  rsqrt is computed as exp(-0.5*ln(y))
    so every scalar-engine activation (Exp/Ln/Copy) lives in ONE hardware
    table set and no table reloads occur.
  - RoPE via a single cross-partition stream_shuffle: the head dim is laid
    out host-side so each rope partner pair sits 16 partitions apart inside
    one 32-partition quadrant; the sign-folded sin table makes
    out = x*cos + shuffle(x)*sin exact.
  - Causal attention per head in S^T = [s, t] layout.  ||q||=||k||=sqrt(D)
    after rmsnorm, so scores are bounded and softmax needs no max
    subtraction.  exp runs on psum score pairs; only true diagonal
    128x128 blocks are masked, by one triangular table (fully-masked
    score columns left of each s-chunk's first valid t-block are not
    even computed).
  - AV runs TRANSPOSED: for each 128-wide t-block, matmul(lhsT=es_block,
    rhs=v_chunk) accumulates [t, d+1] in psum with N=65 per matmul --
    about half the tensor-engine streaming cost of the [d, t] form.  The
    ones column of V gives the softmax denominator; the division is a
    per-partition reciprocal + broadcast multiply during psum evacuation.
  - ONE AllToAll (8 cores) swaps head-shards for T-shards in [t, h, d]
    layout; each core then DMA-TRANSPOSES (xbar dma transpose, ~14ns/tile)
    its received blocks into [hd, t] tiles and runs the wo projection with
    no further reduction.

Matmul operands are float16 (fp32 PSUM accumulation).
"""

import numpy as np

import concourse.bass as bass
import concourse.mybir as mybir
import concourse.tile as tile
from concourse import bacc
from concourse.bass_utils import run_bass_kernel_spmd

N_CORES = 8
B, T, C = 2, 2048, 1024
H, D = 16, 64          # global heads, head dim
HL = 4                 # heads per core
ROPE_THETA = 10000.0
EPS = 1e-6
KO = C // 128          # contraction chunks for C
NT = T // 512          # 512-wide t-chunks
NS = T // 128          # 128-wide s-chunks
TS = T // N_CORES      # t-rows owned per core after AllToAll (256)

F16 = mybir.dt.float16
F32 = mybir.dt.float32

_BUILD_CACHE = {}
SKIP_COLLECTIVE = False  # drop the AllToAll (for single-core TimelineSim)
N_COLLECTIVES = 1

# rope partner shuffle: swap 16-partition halves within each 32-partition
# quadrant (see _host_inputs for the matching weight-row interleave)
SWAP16 = [(i + 16) % 32 for i in range(32)]


def _build_program():
    nc = bacc.Bacc(
        "TRN2",
        target_bir_lowering=False,
        debug=False,
        enable_asserts=False,
        num_devices=N_CORES,
    )
    xT = nc.dram_tensor("xT", [C, T], F16, kind="ExternalInput").ap()
    wqkvT = nc.dram_tensor("wqkvT", [C, 3 * HL * D], F16, kind="ExternalInput").ap()
    woT = nc.dram_tensor("woT", [H * D, C], F16, kind="ExternalInput").ap()
    # rope tables: [:, 0] = cos, [:, 1] = sign-folded sin
    cossin = nc.dram_tensor("cossin", [128, 2, T], F16, kind="ExternalInput").ap()
    # ln scale/bias with the norm weight folded in ([:, 0:2] = scale q/k,
    # [:, 2:4] = bias q/k):
    # exp(-0.5*ln(sum*qkw_s + qkw_b)) == w * rsqrt(mean + eps) for w > 0
    qkw = nc.dram_tensor("qkw", [128, 4], F32, kind="ExternalInput").ap()
    # [:, 0:128] block-diag ones (rmsnorm sum), [:, 128:256] causal triangle
    tables = nc.dram_tensor("tables", [128, 256], F16, kind="ExternalInput").ap()
    out = nc.dram_tensor("out", [B, TS, C], F16, kind="ExternalOutput").ap()

    with tile.TileContext(nc) as tc:
        _emit(tc, xT, wqkvT, woT, cossin, qkw, tables, out)
    nc.compile()
    return nc


def _emit(tc, xT, wqkvT, woT, cossin, qkw, tables, out):
    nc = tc.nc
    Exp = mybir.ActivationFunctionType.Exp
    Ln = mybir.ActivationFunctionType.Ln

    import contextlib

    with contextlib.ExitStack() as ctx:
        const = ctx.enter_context(tc.tile_pool(name="const", bufs=1))
        work = ctx.enter_context(tc.tile_pool(name="work", bufs=6))
        work3 = ctx.enter_context(tc.tile_pool(name="work3", bufs=8))
        espool = ctx.enter_context(tc.tile_pool(name="espool", bufs=20))
        ps512 = ctx.enter_context(tc.tile_pool(name="ps512", bufs=2, space="PSUM"))
        pss2 = ctx.enter_context(tc.tile_pool(name="pss2", bufs=2, space="PSUM"))
        po4p = ctx.enter_context(tc.tile_pool(name="po4p", bufs=2, space="PSUM"))
        dram = ctx.enter_context(tc.tile_pool(name="dram", bufs=1, space="DRAM"))

        # ---- persistent SBUF tiles ----
        xT_sb = const.tile([128, KO, T], F16, tag="xT_sb")
        wqkvT_sb = const.tile([128, KO, 3 * HL * D], F16, tag="wqkvT_sb")
        woT_sb = const.tile([128, KO, C], F16, tag="woT_sb")
        cossin_sb = const.tile([128, 2, T], F16, tag="cossin_sb")
        qkw_sb = const.tile([128, 4], F32, tag="qkw_sb")
        tables_sb = const.tile([128, 256], F16, tag="tables_sb")
        cos2_sb = cossin_sb[:, 0, :]
        sin2_sb = cossin_sb[:, 1, :]
        ones_sb = tables_sb[:, 0:128]
        mtri_sb = tables_sb[:, 128:256]
        qT = [
            const.tile([128, T], F16, tag=f"qT{p}", name=f"qT{p}") for p in range(2)
        ]
        kT = [
            const.tile([128, T], F16, tag=f"kT{p}", name=f"kT{p}") for p in range(2)
        ]
        vt = const.tile([128, NS, HL * (D + 1)], F16, tag="vt")
        # one aT tile per (batch, t-slot) so each out-projection group only
        # depends on its own 4 transposes, not all 16
        aTq = [
            const.tile([128, KO, 128], F16, tag=f"aTq{i}", name=f"aTq{i}")
            for i in range(4)
        ]

        # collective buffer in [t, h, d] layout:
        # (dst core, t-slot of 128, 128 t, 4 heads, 64 d)
        cc = dram.tile([N_CORES, 2, 128, HL, D], F16, tag="cc")
        cc_o = dram.tile([N_CORES, 2, 128, HL, D], F16, tag="cc_o")

        # ---- input DMAs (batched to keep HWDGE serialization low; the
        # first 512 t-columns of x and the qkv weights come first, small
        # tables next, then the rest of x; woT last -- it is only needed
        # by the output projection) ----
        xT_r = xT.rearrange("(ko p) t -> p ko t", p=128)
        wq_r = wqkvT.rearrange("(ko p) o -> p ko o", p=128)
        # first 512 t-columns of x and the weights in 2-ko chunks so the
        # in-order PE can start the first projection accumulation while the
        # rest streams in
        for koq in range(4):
            ks = bass.ts(koq, 2)
            nc.sync.dma_start(xT_sb[:, ks, 0:512], xT_r[:, ks, 0:512])
            nc.sync.dma_start(wqkvT_sb[:, ks, :], wq_r[:, ks, :])
        nc.sync.dma_start(qkw_sb[:], qkw[:])
        nc.sync.dma_start(tables_sb[:], tables[:])
        nc.sync.dma_start(cossin_sb[:, :, 0:512], cossin[:, :, 0:512])
        for koh in range(2):
            ks = bass.ts(koh, 4)
            nc.sync.dma_start(xT_sb[:, ks, 512:1024], xT_r[:, ks, 512:1024])
        nc.sync.dma_start(cossin_sb[:, :, 512:T], cossin[:, :, 512:T])
        for koh in range(2):
            ks = bass.ts(koh, 4)
            nc.sync.dma_start(xT_sb[:, ks, 1024:T], xT_r[:, ks, 1024:T])
        nc.sync.dma_start(woT_sb[:], woT.rearrange("(ko p) e -> p ko e", p=128))

        # ones column in the V tiles (drives the softmax denominator)
        nc.vector.memset(
            vt[:].rearrange("p s (h c) -> p s h c", c=D + 1)[:, :, :, D : D + 1], 1.0
        )

        # Load activation-table set 6 (natural_log_exp_and_others) ONCE up
        # front: it contains every function this kernel uses (Exp, Ln, Copy,
        # Square), so the Bacc fixpoint pass never needs to insert another
        # (it would otherwise thrash between the ln-only and exp-only sets).
        nc.scalar.add_instruction(
            mybir.InstLoadActFuncSet(
                name=nc.get_next_instruction_name(),
                ins=[],
                outs=[],
                act_func_set_id=6,
            )
        )

        # ---- thunk machinery: the PE executes strictly in order, so small
        # PE work units (projection ko-chunks, the norm ones-matmul) are
        # queued and popped between attention pairs.  This fills the PE
        # while the scalar engine's exp gates each pair's AV matmuls, and
        # it naturally delays each norm matmul until its DVE-side sum of
        # squares is long done. ----
        from collections import deque

        # entries are (deadline_key, thunk); deadline_key = (ti, si) means
        # the thunk MUST have run before attention chunk `ti` emits the pair
        # containing s-chunk `si` (si = -1: before chunk ti's first pair).
        # Pops are allowed any time; flush_until forces overdue thunks.
        filler = deque()

        def pop_filler(n=1):
            for _ in range(n):
                if filler:
                    filler.popleft()[1]()

        def flush_until(key):
            while filler and filler[0][0] <= key:
                filler.popleft()[1]()

        def flush_filler():
            while filler:
                filler.popleft()[1]()

        def qk_thunks(ot, ti):
            """q/k projection + rmsnorm + rope for one head-pair o-tile and
            one 512-wide t-chunk, split into 3 PE-granular thunks."""
            is_q = ot < 2
            pair = ot % 2
            wcol = 0 if is_q else 1
            dest = qT[pair] if is_q else kT[pair]
            tsl = bass.ts(ti, 512)
            st = {}

            def mk_proj(k0):
                def tp():
                    if k0 == 0:
                        st["ps"] = ps512.tile([128, 512], F32, tag="ps512", name="ps")
                    ps = st["ps"]
                    for ko in range(k0, k0 + 2):
                        nc.tensor.matmul(
                            ps[:],
                            lhsT=wqkvT_sb[:, ko, bass.ts(ot, 128)],
                            rhs=xT_sb[:, ko, tsl],
                            start=(ko == 0),
                            stop=(ko == KO - 1),
                        )
                    if k0 + 2 == KO:
                        # Evacuate psum immediately (frees the bank for the
                        # next accumulation), run the chain from the f16 copy.
                        rawe = work3.tile([128, 512], F16, tag="rawe", name="rawe")
                        nc.vector.tensor_copy(rawe[:], ps[:])
                        # rope partner shuffle does not depend on the norm
                        # coefficient (cb is constant across partner rows, so
                        # shuffle(raw*cb) == shuffle(raw)*cb and cb folds
                        # into the cos/sin coefficient tiles in t3).

                        sq = work3.tile([128, 512], F16, tag="sq", name="sq")
                        nc.vector.tensor_mul(sq[:], rawe[:], rawe[:])
                        st["rawe"], st["sq"] = rawe, sq

                return tp

            def t3():
                # Per-head sum of squares, replicated across the head's 64
                # partitions by the block-diagonal ones lhsT in one matmul.
                # The norm weight is folded into the ln's per-partition
                # scale/bias; rsqrt(y) = exp(-0.5*ln(y)) keeps the scalar
                # engine inside ONE activation-table set (Exp/Ln/Copy).
                ms = ps512.tile([128, 512], F32, tag="ps512", name="ms")
                nc.tensor.matmul(
                    ms[:], lhsT=ones_sb[:], rhs=st["sq"][:], start=True, stop=True
                )
                lncb = work.tile([128, 512], F32, tag="lncb", name="lncb")
                nc.scalar.activation(
                    lncb[:],
                    ms[:],
                    Ln,
                    bias=qkw_sb[:, 2 + wcol : 3 + wcol],
                    scale=qkw_sb[:, wcol : wcol + 1],
                )
                cb = work.tile([128, 512], F16, tag="cb", name="cb")
                nc.scalar.activation(cb[:], lncb[:], Exp, scale=-0.5)
                # dest = rawe*(cb*cos) + shuffle(rawe)*(cb*sin)
                rawn = work.tile([128, 512], F16, tag="rawn", name="rawn")
                nc.vector.tensor_mul(rawn[:], st["rawe"][:], cb[:])
                rsw = work.tile([128, 512], F16, tag="rsw", name="rsw")
                nc.vector.stream_shuffle(rsw[:], rawn[:], SWAP16)
                rot = work.tile([128, 512], F16, tag="rot", name="rot")
                nc.vector.tensor_mul(rot[:], rsw[:], sin2_sb[:, tsl])
                cosm = work.tile([128, 512], F16, tag="cosm", name="cosm")
                nc.vector.tensor_mul(cosm[:], rawn[:], cos2_sb[:, tsl])
                nc.vector.tensor_add(dest[:, tsl], cosm[:], rot[:])

            return [mk_proj(0), mk_proj(2), mk_proj(4), mk_proj(6), t3]

        def v_thunks(st_):
            vt_heads = vt[:].rearrange("p s (h c) -> p s h c", c=D + 1)
            state = {}

            def mk_v(k0):
                def tv():
                    if k0 == 0:
                        state["psv"] = ps512.tile(
                            [128, 512], F32, tag="ps512", name="psv"
                        )
                    psv = state["psv"]
                    for ko in range(k0, k0 + 4):
                        nc.tensor.matmul(
                            psv[:, 0 : HL * D],
                            lhsT=xT_sb[:, ko, bass.ts(st_, 128)],
                            rhs=wqkvT_sb[:, ko, 2 * HL * D : 3 * HL * D],
                            start=(ko == 0),
                            stop=(ko == KO - 1),
                        )
                    if k0 + 4 == KO:
                        nc.vector.tensor_copy(
                            vt_heads[:, st_, :, 0:D],
                            psv[:, 0 : HL * D].rearrange("p (h c) -> p h c", c=D),
                        )

                return tv

            return [mk_v(0), mk_v(4)]

        def emit_attn(h, ti):
            """attention for one head and one 512-wide t-chunk.  Scores come
            in s-chunk pairs sharing a [128, 2, 512] psum tile (one exp per
            pair); AV accumulates transposed [t, d+1] tiles per 128-wide
            t-block with N=65 matmuls.  The pair loop is software-pipelined:
            pair p's AV runs after pair p+1's QK (plus one filler thunk), so
            the exp latency hides behind real PE work."""
            pair = h // 2
            hrow = bass.ds(64 * (h % 2), 64)
            po4 = po4p.tile([128, 4, D + 1], F32, tag="po4", name="po4")
            n_si = 4 * ti + 4
            av_state = {"first": True}

            def po4_mm(es, u, tj, cut, si, stop):
                nc.tensor.matmul(
                    po4[:, tj, :],
                    lhsT=es[:, u, bass.ds(128 * tj - cut, 128)],
                    rhs=vt[:, si, bass.ds(h * (D + 1), D + 1)],
                    start=av_state["first"],
                    stop=stop,
                )
                av_state["first"] = False

            def emit_av(p):
                # transposed AV: accumulate [t, d+1] per 128-wide t-block.
                # The whole po4 bank is ONE psum accumulation group (psum
                # group start/stop is bank-granular); per-element
                # has_written bits make the first write to each t-block an
                # overwrite and later ones accumulates.  Blocks that need no
                # causal mask go first so only the diagonal blocks wait on
                # the gpsimd mask ops.
                (es, j, cut) = p
                si0 = 4 * ti + j
                for u in range(2):
                    si = si0 + u
                    for tj in range(max(0, j + u + 1), 4):
                        po4_mm(es, u, tj, cut, si, False)
                # mask ONLY the true diagonal 128x128 blocks, on gpsimd
                meng = nc.vector
                for u in range(2):
                    tj = j + u
                    if tj >= 0:
                        blk = es[:, u, bass.ds(128 * tj - cut, 128)]
                        meng.tensor_mul(blk, blk, mtri_sb[:])
                for u in range(2):
                    tj = j + u
                    if tj >= 0:
                        po4_mm(es, u, tj, cut, 4 * ti + j + u, tj == 3)

            pend = []
            for si0 in range(0, n_si, 2):
                flush_until((ti, h, si0 + 1))
                j = si0 - 4 * ti
                # the last (most-diagonal) pair only attends within the
                # upper half of the t-chunk -- the lower half is fully
                # masked zeros, so compute it at half width
                last = j == 2
                tw = 256 if last else 512
                cut = 512 - tw           # es col 0 == chunk-local t = cut
                toff = 512 * ti + cut
                ps2 = pss2.tile([128, 2, 512], F32, tag="pss2", name="ps2")
                for u in range(2):
                    nc.tensor.matmul(
                        ps2[:, u, 0:tw],
                        lhsT=kT[pair][hrow, bass.ts(si0 + u, 128)],
                        rhs=qT[pair][hrow, bass.ds(toff, tw)],
                        start=True,
                        stop=True,
                    )
                es = espool.tile([128, 2, 512], F16, tag="es", name="es")
                nc.scalar.activation(
                    es[:, :, 0:tw], ps2[:, :, 0:tw], Exp, scale=1.0 / np.sqrt(D)
                )
                pend.append((es, j, cut))
                if len(pend) > 1:
                    pop_filler()
                    emit_av(pend.pop(0))
            while pend:
                pop_filler()
                emit_av(pend.pop(0))
            # softmax division on evacuation: per-partition reciprocal of
            # the ones-column, broadcast along d
            rec4 = work.tile([128, 4], F32, tag="rec4", name="rec4")
            nc.vector.reciprocal(
                rec4[:], po4[:, :, D : D + 1].rearrange("p a b -> p (a b)")
            )
            ob = work.tile([128, 4, D], F16, tag="ob", name="ob")
            nc.vector.tensor_mul(
                ob[:], po4[:, :, 0:D], rec4[:].broadcast_to([128, 4, D])
            )
            # one DMA: t-block tj goes to dst core (4*ti+tj)//2, slot
            # (4*ti+tj)%2 -- contiguous (dst, slot) blocks in cc.  Issued
            # via the gpsimd SWDGE queue: the SP queue's DMA counting
            # semaphore then never chains the tail transposes behind these
            # writes.
            ccr = cc[:].rearrange("dst slot t h d -> t (dst slot) h d")
            nc.sync.dma_start(ccr[:, bass.ds(4 * ti, 4), h, :], ob[:])

        def emit_a2a():
            if SKIP_COLLECTIVE:
                return
            nc.gpsimd.collective_compute(
                "AllToAll",
                mybir.AluOpType.bypass,
                replica_groups=[list(range(N_CORES))],
                ins=[cc.opt()],
                outs=[cc_o.opt()],
            )

        def emit_tail():
            # aT column b*KO+ko holds hd rows [128*ko, 128*ko+128) of batch
            # b = global heads {2*ko, 2*ko+1}: source core 4*b + g covers
            # columns b*KO + {2g, 2g+1}.  One xbar dma transpose per
            # (source core, t-slot) turns the received [128 t, 4 h, 64 d]
            # block into both aT column tiles at once; the out projection
            # for that (b, t-slot) follows immediately so PE work overlaps
            # the remaining transposes.  In SKIP_COLLECTIVE timing mode the
            # transposes read cc itself, which models the true
            # after-all-attention ordering without the collective.
            src_buf = cc if SKIP_COLLECTIVE else cc_o
            for b in range(B):
                for slot in range(2):
                    aT = aTq[2 * b + slot]
                    for g in range(4):
                        nc.sync.dma_start_transpose(
                            aT[:, bass.ds(2 * g, 2), :],
                            src_buf[4 * b + g, slot, :, :, :],
                        )
                    tt = slot
                    for ec in range(C // 512):
                        pout = ps512.tile([128, 512], F32, tag="ps512", name="pout")
                        for ko in range(KO):
                            nc.tensor.matmul(
                                pout[:],
                                lhsT=aT[:, ko, :],
                                rhs=woT_sb[:, ko, bass.ts(ec, 512)],
                                start=(ko == 0),
                                stop=(ko == KO - 1),
                            )
                        ob = work.tile([128, 512], F16, tag="obp", name="obp")
                        nc.vector.tensor_copy(ob[:], pout[:])
                        nc.sync.dma_start(
                            out[b, bass.ts(tt, 128), bass.ts(ec, 512)], ob[:]
                        )

        # ---- emission order: chunk-0 projections up front (norm matmuls
        # staggered one slot behind their projections), then per 512-wide
        # t-chunk the NEXT chunk's projection thunks are queued and popped
        # between attention pairs; then ONE AllToAll, the dma-transposed
        # aT loads, and the output projection ----
        ot_order = (2, 3, 0, 1)
        pre = [qk_thunks(ot, 0) for ot in ot_order]
        vpre = [v_thunks(st) for st in range(4)]
        for i in range(4):
            for tp in pre[i][:4]:
                tp()
            if i > 0:
                pre[i - 1][4]()
            vpre[i][0]()
            vpre[i][1]()
        pre[3][4]()
        for ti in range(NT):
            # queue the next chunk's projection thunks with deadlines;
            # attention pairs pop one between each score/AV step so the PE
            # never idles while the scalar engine's exp stream gates the
            # AVs.  Q tiles are due at the next chunk's first pair, but K
            # tiles and V chunks are only due at the pair that first reads
            # them -- so they spill INTO the next chunk's attention and keep
            # feeding the PE during the exp-bound final chunk.
            if ti + 1 < NT:
                t1 = ti + 1
                # stagger each o-tile's norm/rope thunk (t3, whose ms matmul
                # waits on the DVE sum-of-squares) one slot behind its
                # projections, so a boundary flush never stalls the in-order
                # PE at an ms matmul
                qk_lists = [qk_thunks(ot_order[h], t1) for h in range(HL)]
                v_lists = [v_thunks(4 * t1 + h) for h in range(HL)]
                for h in range(HL):
                    filler.extend(((t1, 0, -1), t) for t in qk_lists[h][:4])
                    if h > 0:
                        filler.append(((t1, 0, -1), qk_lists[h - 1][4]))
                    filler.extend(((t1, 0, -1), t) for t in v_lists[h])
                filler.append(((t1, 0, -1), qk_lists[HL - 1][4]))
            for h in range(HL):
                emit_attn(h, ti)
            flush_filler()
        emit_a2a()
        emit_tail()


def _host_inputs(x, wqkv, wo, q_norm_w, k_norm_w):
    """Build the per-core input maps (all host-side prep is layout/dtype only)."""
    x = np.asarray(x, dtype=np.float32)
    wqkv = np.asarray(wqkv, dtype=np.float32)
    wo = np.asarray(wo, dtype=np.float32)
    q_norm_w = np.asarray(q_norm_w, dtype=np.float32)
    k_norm_w = np.asarray(k_norm_w, dtype=np.float32)

    # head-dim interleave: rope partner pairs (x1_i, x2_i) sit 16 partitions
    # apart within one 32-partition quadrant, so ONE stream_shuffle (swap
    # 16-halves per quadrant) aligns every partner.  perm64[new] = old.
    perm64 = np.concatenate(
        [np.arange(0, 16), np.arange(32, 48), np.arange(16, 32), np.arange(48, 64)]
    )
    # rope frequency index and partner-sign per (new) position
    j = np.arange(64)
    q32, r32 = j // 32, j % 32
    freq_idx = 16 * q32 + (r32 % 16)          # pair index i in [0, 32)
    is_x1 = r32 < 16                          # rows holding x1_i

    inv_freq = (
        1.0 / (ROPE_THETA ** (np.arange(0, D, 2, dtype=np.float32) / D))
    ).astype(np.float32)
    freqs = np.arange(T, dtype=np.float32)[:, None] * inv_freq[None, :]  # [T, 32]
    cosT = np.cos(freqs).astype(np.float32)  # [T, 32]
    sinT = np.sin(freqs).astype(np.float32)
    # per-partition tables for a 2-head (128-row) o-tile
    cos64 = cosT[:, freq_idx].T              # [64, T]
    sgn = np.where(is_x1, -1.0, 1.0).astype(np.float32)
    sin64 = (sinT[:, freq_idx] * sgn[None, :]).T
    cossin = np.stack(
        [np.tile(cos64, (2, 1)), np.tile(sin64, (2, 1))], axis=1
    ).astype(np.float16)  # [128, 2, T]

    qw2 = np.concatenate([q_norm_w[perm64], q_norm_w[perm64]])  # [128]
    kw2 = np.concatenate([k_norm_w[perm64], k_norm_w[perm64]])
    qkw = np.stack(
        [
            1.0 / (D * qw2 * qw2),
            1.0 / (D * kw2 * kw2),
            EPS / (qw2 * qw2),
            EPS / (kw2 * kw2),
        ],
        axis=1,
    ).astype(np.float32)  # [128, 4] = scale q/k, bias q/k

    tables = np.zeros((128, 256), dtype=np.float16)
    tables[0:64, 0:64] = 1.0
    tables[64:128, 64:128] = 1.0
    # causal mask for a diagonal 128x128 block: valid iff s <= t
    s_i = np.arange(128)[:, None]
    t_i = np.arange(128)[None, :]
    tables[:, 128:256] = (s_i <= t_i).astype(np.float16)

    woT = np.ascontiguousarray(wo.T).astype(np.float16)  # [hd, e]

    xT_b = [np.ascontiguousarray(x[b].T).astype(np.float16) for b in range(B)]

    # per-head row interleave for the q and k blocks of wqkv
    perm256 = np.concatenate([64 * hh + perm64 for hh in range(HL)])

    in_maps = []
    for c in range(N_CORES):
        b, g = c // 4, c % 4
        rq = slice(256 * g, 256 * g + 256)
        wsel = np.concatenate(
            [wqkv[rq][perm256], wqkv[C:][rq][perm256], wqkv[2 * C :][rq]], axis=0
        )  # [768, C]
        wqkvT = np.ascontiguousarray(wsel.T).astype(np.float16)
        in_maps.append(
            {
                "xT": xT_b[b],
                "wqkvT": wqkvT,
                "woT": woT,
                "cossin": cossin,
                "qkw": qkw,
                "tables": tables,
            }
        )
    return in_maps


def get_program():
    if "nc" not in _BUILD_CACHE:
        _BUILD_CACHE["nc"] = _build_program()
    return _BUILD_CACHE["nc"]


def kernel(x, wqkv, wo, q_norm_w, k_norm_w):
    nc = get_program()
    in_maps = _host_inputs(x, wqkv, wo, q_norm_w, k_norm_w)
    res = run_bass_kernel_spmd(nc, in_maps, core_ids=list(range(N_CORES)))
    full = np.empty((B, T, C), dtype=np.float32)
    for c in range(N_CORES):
        o = res.results[c]["out"]  # [B, TS, C] f16
        full[:, TS * c : TS * (c + 1), :] = np.asarray(o, dtype=np.float32)
    return full
